# revision 1
# baseline (speedup 1.0000x reference)
"""EquivariantCrossAttention Trainium2 kernel (8 NeuronCores, SPMD) — v2.

kernel(**inputs) takes the FULL unsharded inputs from reference's
setup_inputs() and returns the FULL (B, N, DH) float32 output.

Sharding: flattened query axis (B*N = 4096) split into 8 shards of 512
queries; core i gets queries [512*i, 512*(i+1)) plus its batch's latent
tables. Weights replicated.

Hardcoded problem shapes: B=2 N=2048 L=1024 K=16 CD=2 H=4 DH=128 HD=512.

Final: 749.6us HW max-core / 743.1us mean (baseline 995us), rel err 8.5e-3 (gate 2e-2).
PSUM tags: mm bufs=4 (deeper matmul-group pipelining), stat bufs=2, tr bufs=2.

v2 changes vs v1 (995us):
  - stage-major per 128-query tile: all same-table-set ScalarE activations
    batch together (6 ACT table loads per tile instead of ~35)
  - ONE combined dma_gather per tile (2048 rows x 768B: c | pb_frac f16 |
    misc f16 with 1/sigma^2) instead of 8 gathers of 1280B total
  - RFF via frac trick in f16: t_x pre-frac'd per tile, pb stored as
    frac(p@Bs) f16 (no split-bf16 hi/lo); cos(2pi t) = Sin(pi/2 - 2pi|f|)
    via one abs_max tensor_scalar (kills the +0.25/int-cast chain)
  - sins halved: sin(fs) and sin(pi/2-2pi|fs|) computed on all 128
    partitions at once; eq_w1/ev_w1 split into sin/cos halves host-side
  - s3 (mean-correction) folded into the output matmul as rank-1 terms
  - per-tile [128,2048] DVE/ACT ops instead of per-chunk [128,512] where
    PSUM allows (amortizes the SBUF read-write bubble)
  - software-pipelined tiles: head(t+2) (scores/topk/gather) emitted
    before body(t+1) so gathers and PE score matmuls overlap tile bodies
"""

import sys

sys.path.insert(0, "/opt/trn_rl_repo")

import numpy as np
import ml_dtypes

import concourse.bass as bass
import concourse.bacc as bacc
import concourse.mybir as mybir
import concourse.tile as tile
from concourse.masks import make_identity

F32 = mybir.dt.float32
BF16 = mybir.dt.bfloat16
F16 = mybir.dt.float16
U32 = mybir.dt.uint32
I32 = mybir.dt.int32
I16 = mybir.dt.int16
AF = mybir.ActivationFunctionType
OP = mybir.AluOpType
AX = mybir.AxisListType

B, N, L, K, CD, H, DH, D = 2, 2048, 1024, 16, 2, 4, 128, 128
HD = H * DH
FQ = 2.0
FV = 2.0
SCALE = 1.0 / float(np.sqrt(DH))
NCORES = 8
NQ = (B * N) // NCORES  # queries per core = 512
QT = NQ // 128  # query tiles per core = 4
NCH = 4  # chunks per query tile
CQ = 128 // NCH  # queries per chunk = 32
CR = CQ * K  # rows per chunk = 512
TR = 128 * K  # rows per tile = 2048
GELU = AF.Gelu_apprx_tanh
TWO_PI = 2.0 * np.pi

WSPECS = [
    ("rff", [CD, 128], F32),
    ("ws1", [128, 128], BF16),   # rows 0:64 eq_w1 sin-half, 64:128 ev_w1 sin-half
    ("wc1", [128, 128], BF16),   # rows 0:64 eq_w1 cos-half, 64:128 ev_w1 cos-half
    ("eq_b1", [128, 1], F32),
    ("Mq", [128, 512], BF16),
    ("w1v", [1, 512], BF16),
    ("w2v", [128, 4], BF16),
    ("attconst", [128, 1], F32),
    ("ev_b1", [128, 1], F32),
    ("ev_w2", [128, 128], BF16),
    ("ev_b2", [128, 1], F32),
    ("ivw1", [128, 128], BF16),
    ("ivb1", [128, 1], F32),
    ("ivw2g", [128, 512], BF16),
    ("wv", [128, 512], BF16),
    ("WA", [128, 512], BF16),
    ("WB", [128, 512], BF16),
    ("mw1", [128, 128], BF16),
    ("mb1p", [128, 4], F32),
    ("Wmo", [128, 512], BF16),
    ("wmo1m", [128, 512], BF16),  # row 32h, cols 128h:128h+128 = -(Wmo_h^T @ 1)
    ("bmo", [128, 1], F32),
    ("cw1", [128, 128], BF16),
    ("cb1", [128, 1], F32),
    ("cw2g", [128, 128], BF16),
    ("cw2b", [128, 128], BF16),
    ("cb2g1", [128, 1], F32),
    ("cb2b", [128, 1], F32),
]


def _bcast_inner(ap, n):
    """[.., Q] AP -> [.., Q, n] with a stride-0 inner dim (free broadcast)."""
    newap = [list(p) for p in ap.ap] + [[0, n]]
    return bass.AP(ap.tensor, ap.offset, newap)


def build_program():
    nc = bacc.Bacc()

    x_d = nc.declare_dram_parameter("x", [NQ, CD], F32, isOutput=False)
    xh_d = nc.declare_dram_parameter("xh", [NQ, DH], F32, isOutput=False)
    gtbl_d = nc.declare_dram_parameter("gtbl", [L, 3 * D], BF16, isOutput=False)
    p2t_d = nc.declare_dram_parameter("p2t", [CD, L], F32, isOutput=False)
    npsq_d = nc.declare_dram_parameter("npsq", [1, L], F32, isOutput=False)
    w_d = {}
    for name, shape, dt in WSPECS:
        w_d[name] = nc.declare_dram_parameter(name, shape, dt, isOutput=False)
    out_d = nc.declare_dram_parameter("out", [NQ, DH], F32, isOutput=True)

    with tile.TileContext(nc) as tc:
        _emit(nc, tc, x_d, xh_d, gtbl_d, p2t_d, npsq_d, w_d, out_d)
    nc.compile()
    return nc


def _emit(nc, tc, x_d, xh_d, gtbl_d, p2t_d, npsq_d, w_d, out_d):
    const = tc.alloc_tile_pool(name="const", bufs=1)
    wpool = tc.alloc_tile_pool(name="wpool", bufs=1)
    core = tc.alloc_tile_pool(name="core", bufs=1)
    hd = tc.alloc_tile_pool(name="hd", bufs=1)   # per-tile head tiles
    tl = tc.alloc_tile_pool(name="tl", bufs=1)   # per-tile body tiles
    ck = tc.alloc_tile_pool(name="ck", bufs=1)   # per-chunk transient tiles
    psp = tc.alloc_tile_pool(name="psp", bufs=1, space="PSUM")
    _pools = [const, wpool, core, hd, tl, ck, psp]

    _psn = [0]

    def PS(shape, tag, bufs, dtype=F32):
        _psn[0] += 1
        return psp.tile(
            shape, dtype, space="PSUM", tag=tag, bufs=bufs, name=f"ps_{tag}_{_psn[0]}"
        )

    # ---------- constants ----------
    ident = const.tile([128, 128], F32)
    make_identity(nc, ident[:])

    ones_col_bf = const.tile([128, 1], BF16)
    nc.vector.memset(ones_col_bf[:], 1.0)
    inv128_bf = const.tile([128, 1], BF16)
    nc.vector.memset(inv128_bf[:], 1.0 / 128.0)
    ones_row_bf = const.tile([1, 128], BF16)
    nc.vector.memset(ones_row_bf[:], 1.0)
    ones_row512_bf = const.tile([1, 512], BF16)
    nc.vector.memset(ones_row512_bf[:], 1.0)
    half_row128_bf = const.tile([1, 128], BF16)
    nc.vector.memset(half_row128_bf[:], 0.5)
    onesmat_bf = const.tile([128, 128], BF16)
    nc.vector.memset(onesmat_bf[:], 1.0)
    zeros_row_bf = const.tile([1, 128], BF16)
    nc.vector.memset(zeros_row_bf[:], 0.0)
    zeros_row512_bf = const.tile([1, 512], BF16)
    nc.vector.memset(zeros_row512_bf[:], 0.0)
    ones_row_f32 = const.tile([1, 128], F32)
    nc.vector.memset(ones_row_f32[:], 1.0)
    pihalf = const.tile([128, 1], F32)
    nc.vector.memset(pihalf[:], np.pi / 2.0)
    eps_col = const.tile([128, 1], F32)
    nc.vector.memset(eps_col[:], 1e-6)

    # ---------- weights ----------
    W = {}
    for name, shape, dt in WSPECS:
        wt = wpool.tile(shape, dt, name=f"w_{name}", tag=f"w_{name}")
        nc.sync.dma_start(out=wt[:], in_=w_d[name][:])
        W[name] = wt

    def Wh(name, h, w=128):
        return W[name][:, h * w : (h + 1) * w]

    # ---------- per-core precompute ----------
    x_sb = core.tile([128, QT, CD], F32)
    nc.sync.dma_start(out=x_sb[:], in_=x_d[:].rearrange("(t q) c -> q t c", q=128))
    xsq = core.tile([128, QT], F32)
    xs2 = core.tile([128, QT, CD], F32)
    nc.vector.tensor_tensor(out=xs2[:], in0=x_sb[:], in1=x_sb[:], op=OP.mult)
    nc.vector.tensor_reduce(out=xsq[:], in_=xs2[:], axis=AX.X, op=OP.add)

    x_fm = core.tile([CD, NQ], F32)
    for t in range(QT):
        tp = PS([CD, 128], "tr", 2)
        nc.tensor.transpose(out=tp[:], in_=x_sb[:, t, :], identity=ident[:])
        nc.vector.tensor_copy(out=x_fm[:, 128 * t : 128 * (t + 1)], in_=tp[:])

    p2_fm = core.tile([CD, L], F32)
    nc.sync.dma_start(out=p2_fm[:], in_=p2t_d[:])
    npsq = core.tile([1, L], F32)
    nc.sync.dma_start(out=npsq[:], in_=npsq_d[:])

    # ---------- cFFN on x_h (512 queries at once) ----------
    xh_fm = core.tile([128, NQ], BF16)
    xh_rm = core.tile([128, QT, DH], F32)
    nc.sync.dma_start(out=xh_rm[:], in_=xh_d[:].rearrange("(t q) c -> q t c", q=128))
    for t in range(QT):
        tp = PS([128, 128], "tr", 2)
        nc.tensor.transpose(out=tp[:], in_=xh_rm[:, t, :], identity=ident[:])
        nc.vector.tensor_copy(out=xh_fm[:, 128 * t : 128 * (t + 1)], in_=tp[:])

    c1ps = PS([128, NQ], "mm", 4)
    nc.tensor.matmul(out=c1ps[:], lhsT=W["cw1"][:], rhs=xh_fm[:], start=True, stop=True)
    c1 = core.tile([128, NQ], BF16)
    nc.scalar.activation(out=c1[:], in_=c1ps[:], func=GELU, bias=W["cb1"][:])

    cmean = PS([128, NQ], "stat", 2)
    nc.tensor.matmul(out=cmean[0:1, :], lhsT=inv128_bf[:], rhs=c1[:], start=True, stop=True)
    c1sq = core.tile([128, NQ], BF16)
    nc.vector.tensor_tensor(out=c1sq[:], in0=c1[:], in1=c1[:], op=OP.mult)
    cmsq = PS([128, NQ], "stat", 2)
    nc.tensor.matmul(out=cmsq[0:1, :], lhsT=inv128_bf[:], rhs=c1sq[:], start=True, stop=True)

    cm2 = core.tile([1, NQ], F32)
    nc.scalar.square(out=cm2[:], in_=cmean[0:1, :])
    cvar = core.tile([1, NQ], F32)
    nc.vector.tensor_tensor(out=cvar[:], in0=cmsq[0:1, :], in1=cm2[:], op=OP.subtract)
    crstd_bf = core.tile([1, NQ], BF16)
    nc.scalar.activation(
        out=crstd_bf[:], in_=cvar[:], func=AF.Abs_reciprocal_sqrt, bias=eps_col[0:1, :]
    )
    cmr_bf = core.tile([1, NQ], BF16)
    nc.vector.tensor_tensor(out=cmr_bf[:], in0=cmean[0:1, :], in1=crstd_bf[:], op=OP.mult)
    crbc = PS([128, NQ], "mm", 4)
    nc.tensor.matmul(out=crbc[:], lhsT=ones_row_bf[:], rhs=crstd_bf[:], start=True, stop=True)
    cmbc = PS([128, NQ], "mm", 4)
    nc.tensor.matmul(out=cmbc[:], lhsT=ones_row_bf[:], rhs=cmr_bf[:], start=True, stop=True)
    z1c = core.tile([128, NQ], BF16)
    nc.vector.tensor_tensor(out=z1c[:], in0=c1[:], in1=crbc[:], op=OP.mult)
    zc = core.tile([128, NQ], BF16)
    nc.vector.tensor_tensor(out=zc[:], in0=z1c[:], in1=cmbc[:], op=OP.subtract)

    gp1 = core.tile([128, NQ], BF16)
    modadd = core.tile([128, NQ], BF16)
    gps_ = PS([128, NQ], "mm", 4)
    nc.tensor.matmul(out=gps_[:], lhsT=W["cw2g"][:], rhs=zc[:], start=True, stop=True)
    nc.scalar.activation(out=gp1[:], in_=gps_[:], func=AF.Identity, bias=W["cb2g1"][:])
    btps = PS([128, NQ], "mm", 4)
    nc.tensor.matmul(out=btps[:], lhsT=W["cw2b"][:], rhs=zc[:], start=True, stop=True)
    bt = core.tile([128, NQ], BF16)
    nc.scalar.activation(out=bt[:], in_=btps[:], func=AF.Identity, bias=W["cb2b"][:])
    ma1 = core.tile([128, NQ], BF16)
    nc.vector.tensor_scalar(
        out=ma1[:], in0=gp1[:], scalar1=W["ev_b2"][:], scalar2=None, op0=OP.mult
    )
    nc.vector.tensor_tensor(out=modadd[:], in0=ma1[:], in1=bt[:], op=OP.add)

    # ================= per query tile: head phase =================
    # scores -> top-16 -> idx16 -> combined gather -> t_x frac
    def emit_head(t):
        qs = slice(128 * t, 128 * (t + 1))

        scores = hd.tile([128, L], F32, tag="scores", bufs=1, name=f"scores_{t}")
        for s in range(2):
            sl = slice(512 * s, 512 * (s + 1))
            scps = PS([128, 512], "mm", 4)
            nc.tensor.matmul(out=scps[:], lhsT=x_fm[:, qs], rhs=p2_fm[:, sl], start=True, stop=False)
            nc.tensor.matmul(out=scps[:], lhsT=ones_row_f32[:], rhs=npsq[:, sl], start=False, stop=True)
            nc.scalar.copy(out=scores[:, sl], in_=scps[:])
        vals = hd.tile([128, K], F32, tag="vals", bufs=2, name=f"vals_{t}")
        idxs = hd.tile([128, K], U32, tag="idxs", bufs=2, name=f"idxs_{t}")
        scr2 = hd.tile([128, L], F32, tag="scr2", bufs=1, name=f"scr2_{t}")
        nc.vector.max(out=vals[:, 0:8], in_=scores[:])
        nc.vector.max_index(out=idxs[:, 0:8], in_max=vals[:, 0:8], in_values=scores[:])
        nc.vector.match_replace(
            out=scr2[:], in_to_replace=vals[:, 0:8], in_values=scores[:], imm_value=-1e30
        )
        nc.vector.max(out=vals[:, 8:16], in_=scr2[:])
        nc.vector.max_index(out=idxs[:, 8:16], in_max=vals[:, 8:16], in_values=scr2[:])

        # index prep for dma_gather: idx16[k, q] (int16) replicated across
        # the 8 gpsimd cores' 16-partition blocks
        idxf = hd.tile([128, K], F32, tag="idxf", bufs=1, name=f"idxf_{t}")
        nc.vector.tensor_copy(out=idxf[:], in_=idxs[:])
        idxt_ps = PS([K, 128], "tr", 2)
        nc.tensor.transpose(out=idxt_ps[:], in_=idxf[:], identity=ident[:])
        idx16 = hd.tile([128, 128], I16, tag="idx16", bufs=2, name=f"idx16_{t}")
        nc.vector.tensor_copy(out=idx16[0:16, :], in_=idxt_ps[:])
        for b in range(1, 8):
            nc.sync.dma_start(out=idx16[16 * b : 16 * b + 16, :], in_=idx16[0:16, :])

        # ONE combined transposed gather for the whole tile (2048 rows):
        # G[128, 0, :] = c (bf16), G[128, 1, :] = frac(p@Bs) (f16 bits),
        # G[0, 2, :] = 1/sigma^2 (f16 bits)
        Gs = []
        for c in range(NCH):
            G = hd.tile([128, 3, CR], BF16, tag=f"G{c}", bufs=2, name=f"G{c}_{t}")
            nc.gpsimd.dma_gather(
                out_ap=G[:], in_ap=gtbl_d[:],
                idxs_ap=idx16[:, 32 * c : 32 * c + 32], num_idxs=CR,
                num_idxs_reg=CR, elem_size=3 * D, transpose=True,
            )
            Gs.append(G)

        # -d^2 row [1, 2048] (q-major) via sbuf->sbuf DMA flatten
        negd2 = hd.tile([128, K], BF16, tag="negd2", bufs=2, name=f"negd2_{t}")
        nc.vector.tensor_scalar(
            out=negd2[:], in0=vals[:], scalar1=xsq[:, t : t + 1], scalar2=None,
            op0=OP.subtract,
        )
        negd2_row = hd.tile([1, TR], BF16, tag="negd2_row", bufs=2, name=f"nd2r_{t}")
        nc.sync.dma_start(out=negd2_row[:], in_=negd2[:])

        # t_x = x @ Bs for this tile, pre-frac'd: txf = t_x - rint(t_x), f16
        txps = PS([128, 128], "tr", 2)
        nc.tensor.matmul(out=txps[:], lhsT=W["rff"][:], rhs=x_fm[:, qs], start=True, stop=True)
        t_x = hd.tile([128, 128], F32, tag="t_x", bufs=1, name=f"t_x_{t}")
        nc.vector.tensor_copy(out=t_x[:], in_=txps[:])
        txi = hd.tile([128, 128], I32, tag="txi", bufs=1, name=f"txi_{t}")
        nc.vector.tensor_copy(out=txi[:], in_=t_x[:])
        txf = hd.tile([128, 128], F32, tag="txf", bufs=2, name=f"txf_{t}")
        nc.vector.tensor_tensor(out=txf[:], in0=t_x[:], in1=txi[:], op=OP.subtract)
        return dict(Gs=Gs, negd2_row=negd2_row, txf=txf)

    # ================= per query tile: body phase =================
    def emit_body(t, ht):
        Gs = ht["Gs"]
        negd2_row = ht["negd2_row"]
        txf = ht["txf"]

        def cg(c):
            return Gs[c][:, 0, :]

        # ---- P2: RFF frac chain (DVE, full tile in 4 chunk writes) ----
        tfull = tl.tile([128, TR], F32, tag="tfull", name=f"tfull_{t}")
        for c in range(NCH):
            cs = slice(CR * c, CR * (c + 1))
            nc.vector.tensor_tensor(
                out=tfull[:, cs].rearrange("p (a b) -> p a b", a=CQ),
                in0=_bcast_inner(txf[:, CQ * c : CQ * (c + 1)], K),
                in1=Gs[c][:, 1, :].bitcast(F16).rearrange("p (a b) -> p a b", a=CQ),
                op=OP.subtract,
            )
        ti = tl.tile([128, TR], I32, tag="ti", name=f"ti_{t}")
        nc.vector.tensor_copy(out=ti[:], in_=tfull[:])
        fs = tl.tile([128, TR], F16, tag="fs", name=f"fs_{t}")
        nc.vector.tensor_tensor(out=fs[:], in0=tfull[:], in1=ti[:], op=OP.subtract)
        fabs = tl.tile([128, TR], F16, tag="fabs", name=f"fabs_{t}")
        nc.scalar.activation(out=fabs[:], in_=fs[:], func=AF.Abs)
        # gw row [1, 2048] = -d^2 * (1/sigma^2)
        gw_row = tl.tile([1, TR], BF16, tag="gw_row", name=f"gw_{t}")
        for c in range(NCH):
            cs = slice(CR * c, CR * (c + 1))
            nc.vector.tensor_tensor(
                out=gw_row[:, cs], in0=negd2_row[:, cs],
                in1=Gs[c][0:1, 2, :].bitcast(F16), op=OP.mult,
            )

        # ---- P3: sins (ScalarE, trig set) ----
        sfull = tl.tile([128, TR], BF16, tag="sfull", name=f"sfull_{t}")
        cfull = tl.tile([128, TR], BF16, tag="cfull", name=f"cfull_{t}")
        nc.scalar.activation(out=sfull[:], in_=fs[:], func=AF.Sin, scale=TWO_PI)
        nc.scalar.activation(
            out=cfull[:], in_=fabs[:], func=AF.Sin, scale=-TWO_PI, bias=pihalf[:]
        )

        # ---- P4: q path + attention logits (gelu set) ----
        g2 = tl.tile([128, TR], BF16, tag="g2", bufs=1, name=f"g2_{t}")
        att_l = tl.tile([128, TR], BF16, tag="att_l", bufs=1, name=f"attl_{t}")
        for c in range(NCH):
            cs = slice(CR * c, CR * (c + 1))
            g2ps = PS([128, CR], "mm", 4)
            nc.tensor.matmul(out=g2ps[:], lhsT=W["ws1"][0:64, :], rhs=sfull[0:64, cs], start=True, stop=False)
            nc.tensor.matmul(out=g2ps[:], lhsT=W["wc1"][0:64, :], rhs=cfull[0:64, cs], start=False, stop=True)
            nc.scalar.activation(out=g2[:, cs], in_=g2ps[:], func=GELU, bias=W["eq_b1"][:])

            attps = PS([128, CR], "stat", 2)
            nc.tensor.matmul(
                out=attps[:], lhsT=half_row128_bf[:],
                rhs=gw_row[:, cs], start=True, stop=False,
                skip_group_check=True,
            )
            for h in range(H):
                ups = PS([128, CR], "mm", 4)
                nc.tensor.matmul(out=ups[:], lhsT=Wh("Mq", h), rhs=g2[:, cs], start=True, stop=False)
                nc.tensor.matmul(
                    out=ups[:], lhsT=W["w1v"][:, 128 * h : 128 * (h + 1)],
                    rhs=ones_row512_bf[:], start=False, stop=True,
                )
                qkp = ck.tile([128, CR], BF16, tag="qkp", bufs=2)
                nc.vector.tensor_tensor(out=qkp[:], in0=ups[:], in1=cg(c), op=OP.mult)
                nc.tensor.matmul(
                    out=attps[32 * h : 32 * h + 1, :], lhsT=ones_col_bf[:], rhs=qkp[:],
                    start=False, stop=False, tile_position=(0, 32 * h),
                    skip_group_check=True,
                )
                nc.tensor.matmul(
                    out=attps[32 * h : 32 * h + 1, :], lhsT=W["w2v"][:, h : h + 1],
                    rhs=g2[:, cs], start=False, stop=(h == H - 1), tile_position=(0, 32 * h),
                    skip_group_check=True,
                )
            # logits to SBUF (Copy is in every ACT table set: no reload)
            nc.scalar.copy(out=att_l[:, cs], in_=attps[:])

        # ---- P5: ev path (gelu set) ----
        iv1 = tl.tile([128, TR], BF16, tag="iv1", bufs=1, name=f"iv1_{t}")
        for c in range(NCH):
            cs = slice(CR * c, CR * (c + 1))
            gq = slice(128 * t + CQ * c, 128 * t + CQ * (c + 1))
            ev1ps = PS([128, CR], "mm", 4)
            nc.tensor.matmul(out=ev1ps[:], lhsT=W["ws1"][64:128, :], rhs=sfull[64:128, cs], start=True, stop=False)
            nc.tensor.matmul(out=ev1ps[:], lhsT=W["wc1"][64:128, :], rhs=cfull[64:128, cs], start=False, stop=True)
            ev1 = ck.tile([128, CR], BF16, tag="ev1", bufs=2)
            nc.scalar.activation(out=ev1[:], in_=ev1ps[:], func=GELU, bias=W["ev_b1"][:])
            ev2ps = PS([128, CR], "mm", 4)
            nc.tensor.matmul(out=ev2ps[:], lhsT=W["ev_w2"][:], rhs=ev1[:], start=True, stop=True)
            mv = ck.tile([128, CQ, K], BF16, tag="mv", bufs=1)
            nc.vector.tensor_tensor(
                out=mv[:], in0=ev2ps[:].rearrange("p (a b) -> p a b", a=CQ),
                in1=_bcast_inner(gp1[:, gq], K), op=OP.mult,
            )
            ivin = ck.tile([128, CQ, K], BF16, tag="ivin", bufs=2)
            nc.vector.tensor_tensor(
                out=ivin[:], in0=mv[:], in1=_bcast_inner(modadd[:, gq], K), op=OP.add
            )
            iv1ps = PS([128, CR], "mm", 4)
            nc.tensor.matmul(
                out=iv1ps[:], lhsT=W["ivw1"][:],
                rhs=ivin[:].rearrange("p a b -> p (a b)"), start=True, stop=True,
            )
            nc.scalar.activation(out=iv1[:, cs], in_=iv1ps[:], func=GELU, bias=W["ivb1"][:])

        # ---- P6: iv LayerNorm (absrsqrt set; square is in-set) ----
        ziv = tl.tile([128, TR], BF16, tag="ziv", bufs=1, name=f"ziv_{t}")
        for c in range(NCH):
            cs = slice(CR * c, CR * (c + 1))
            ivst = PS([128, CR], "stat", 2)
            nc.tensor.matmul(
                out=ivst[0:1, :], lhsT=inv128_bf[:], rhs=iv1[:, cs],
                start=True, stop=True, skip_group_check=True,
            )
            ivsq = ck.tile([128, CR], BF16, tag="ivsq", bufs=2)
            nc.vector.tensor_tensor(out=ivsq[:], in0=iv1[:, cs], in1=iv1[:, cs], op=OP.mult)
            nc.tensor.matmul(
                out=ivst[32:33, :], lhsT=inv128_bf[:], rhs=ivsq[:],
                start=True, stop=True, tile_position=(0, 32),
                skip_group_check=True,
            )
            im2 = ck.tile([1, CR], F32, tag="im2")
            nc.scalar.square(out=im2[:], in_=ivst[0:1, :])
            ivar = ck.tile([1, CR], F32, tag="ivar")
            nc.vector.tensor_tensor(out=ivar[:], in0=ivst[32:33, :], in1=im2[:], op=OP.subtract)
            irstd_bf = ck.tile([1, CR], BF16, tag="irstd_bf")
            nc.scalar.activation(
                out=irstd_bf[:], in_=ivar[:], func=AF.Abs_reciprocal_sqrt,
                bias=eps_col[0:1, :],
            )
            imr_bf = ck.tile([1, CR], BF16, tag="imr_bf")
            nc.vector.tensor_tensor(
                out=imr_bf[:], in0=ivst[0:1, :], in1=irstd_bf[:], op=OP.mult
            )
            irbc = PS([128, CR], "mm", 4)
            nc.tensor.matmul(out=irbc[:], lhsT=ones_row_bf[:], rhs=irstd_bf[:], start=True, stop=True)
            imbc = PS([128, CR], "mm", 4)
            nc.tensor.matmul(out=imbc[:], lhsT=ones_row_bf[:], rhs=imr_bf[:], start=True, stop=True)
            z1 = ck.tile([128, CR], BF16, tag="z1", bufs=1)
            nc.vector.tensor_tensor(out=z1[:], in0=iv1[:, cs], in1=irbc[:], op=OP.mult)
            nc.vector.tensor_tensor(out=ziv[:, cs], in0=z1[:], in1=imbc[:], op=OP.subtract)

        # ---- P7+P8: mFFN per chunk (gelu set, then absrsqrt per chunk) ----
        gms = [
            tl.tile([128, TR], BF16, tag=f"gm{h}", name=f"gm{h}_{t}")
            for h in range(H)
        ]
        mrstd = tl.tile([128, TR], BF16, tag="mrstd", name=f"mrstd_{t}")
        mmr = tl.tile([128, TR], BF16, tag="mmr", name=f"mmr_{t}")
        mvar_all = tl.tile([128, TR], F32, tag="tfull", name=f"mvar_{t}")
        mmean_sb = tl.tile([128, TR], BF16, tag="g2", name=f"mmean_{t}")
        for c in range(NCH):
            cs = slice(CR * c, CR * (c + 1))
            mmean = PS([128, CR], "stat", 2)
            msqp = PS([128, CR], "stat", 2)
            for h in range(H):
                vgps = PS([128, CR], "mm", 4)
                nc.tensor.matmul(out=vgps[:], lhsT=Wh("ivw2g", h), rhs=ziv[:, cs], start=True, stop=True)
                vg_sb = ck.tile([128, CR], BF16, tag="vg_sb", bufs=1)
                nc.vector.tensor_copy(out=vg_sb[:], in_=vgps[:])
                v0ps = PS([128, CR], "mm", 4)
                nc.tensor.matmul(out=v0ps[:], lhsT=Wh("wv", h), rhs=cg(c), start=True, stop=True)
                p_sb = ck.tile([128, CR], BF16, tag="p_sb", bufs=2)
                nc.vector.tensor_tensor(out=p_sb[:], in0=v0ps[:], in1=vg_sb[:], op=OP.mult)
                m1ps = PS([128, CR], "mm", 4)
                nc.tensor.matmul(out=m1ps[:], lhsT=W["mw1"][:], rhs=p_sb[:], start=True, stop=False)
                nc.tensor.matmul(out=m1ps[:], lhsT=Wh("WA", h), rhs=cg(c), start=False, stop=False)
                nc.tensor.matmul(out=m1ps[:], lhsT=Wh("WB", h), rhs=ziv[:, cs], start=False, stop=True)
                nc.scalar.activation(
                    out=gms[h][:, cs], in_=m1ps[:], func=GELU, bias=W["mb1p"][:, h : h + 1]
                )
                nc.tensor.matmul(
                    out=mmean[32 * h : 32 * h + 1, :], lhsT=inv128_bf[:], rhs=gms[h][:, cs],
                    start=True, stop=True, tile_position=(0, 32 * h),
                    skip_group_check=True,
                )
                gsq = ck.tile([128, CR], BF16, tag="gsq", bufs=2)
                nc.vector.tensor_tensor(out=gsq[:], in0=gms[h][:, cs], in1=gms[h][:, cs], op=OP.mult)
                nc.tensor.matmul(
                    out=msqp[32 * h : 32 * h + 1, :], lhsT=inv128_bf[:], rhs=gsq[:],
                    start=True, stop=True, tile_position=(0, 32 * h),
                    skip_group_check=True,
                )
            # evacuate stats; rsqrt deferred to one big op (Copy/Square
            # are in every ACT table set, so no reload mid-gelu-phase)
            mm2 = ck.tile([128, CR], F32, tag="mm2")
            nc.scalar.square(out=mm2[:], in_=mmean[:])
            nc.vector.tensor_tensor(out=mvar_all[:, cs], in0=msqp[:], in1=mm2[:], op=OP.subtract)
            nc.scalar.copy(out=mmean_sb[:, cs], in_=mmean[:])

        # ---- P8b: one deferred rsqrt for the whole tile ----
        nc.scalar.activation(
            out=mrstd[:], in_=mvar_all[:], func=AF.Abs_reciprocal_sqrt, bias=eps_col[:]
        )
        nc.vector.tensor_tensor(out=mmr[:], in0=mmean_sb[:], in1=mrstd[:], op=OP.mult)

        # ---- P9: softmax (exp set) ----
        att_e = tl.tile([128, TR], BF16, tag="att_e", name=f"atte_{t}")
        nc.scalar.activation(out=att_e[:], in_=att_l[:], func=AF.Exp, bias=W["attconst"][:])
        den_t = tl.tile([128, 128], F32, tag="den_t", name=f"den_{t}")
        nc.vector.tensor_reduce(
            out=den_t[:], in_=att_e[:].rearrange("p (a b) -> p a b", a=128),
            axis=AX.X, op=OP.add,
        )
        rden_t = tl.tile([128, 128], F32, tag="rden_t", name=f"rden_{t}")
        nc.vector.reciprocal(out=rden_t[:], in_=den_t[:])
        rdbf = tl.tile([128, 128], BF16, tag="rdbf", name=f"rdbf_{t}")
        nc.vector.tensor_copy(out=rdbf[:], in_=rden_t[:])

        # ---- P10: attention-weighted sums ----
        a2 = tl.tile([128, TR], BF16, tag="a2", name=f"a2_{t}")
        nc.vector.tensor_tensor(out=a2[:], in0=att_e[:], in1=mrstd[:], op=OP.mult)
        a3 = tl.tile([128, TR], BF16, tag="a3", name=f"a3_{t}")
        nc.vector.tensor_tensor(out=a3[:], in0=att_e[:], in1=mmr[:], op=OP.mult)
        s3 = tl.tile([128, 128], F32, tag="s3", name=f"s3_{t}")
        nc.vector.tensor_reduce(
            out=s3[:], in_=a3[:].rearrange("p (a b) -> p a b", a=128), axis=AX.X, op=OP.add
        )
        # s3r = s3 * rden (bf16) -- mean-correction folded into out matmul
        s3rb = tl.tile([128, 128], BF16, tag="s3rb", name=f"s3rb_{t}")
        nc.vector.tensor_tensor(out=s3rb[:], in0=s3[:], in1=rden_t[:], op=OP.mult)

        zaccb = [
            tl.tile([128, 128], BF16, tag=f"zaccb{h}", name=f"zaccb{h}_{t}", bufs=2)
            for h in range(H)
        ]
        for h in range(H):
            zacc = tl.tile([128, 128], F32, tag="zacc", name=f"zacc{h}_{t}")
            for c in range(NCH):
                cs = slice(CR * c, CR * (c + 1))
                qsl = slice(CQ * c, CQ * (c + 1))
                a2bc = PS([128, CR], "mm", 4)
                nc.tensor.matmul(
                    out=a2bc[:], lhsT=onesmat_bf[32 * h : 32 * h + 1, :],
                    rhs=a2[32 * h : 32 * h + 1, cs], start=True, stop=True,
                    tile_position=(32 * h, 0),
                )
                zp = ck.tile([128, CR], BF16, tag="zp", bufs=1)
                nc.vector.tensor_tensor(out=zp[:], in0=gms[h][:, cs], in1=a2bc[:], op=OP.mult)
                nc.vector.tensor_reduce(
                    out=zacc[:, qsl], in_=zp[:].rearrange("p (a b) -> p a b", a=CQ),
                    axis=AX.X, op=OP.add,
                )
            # fold 1/den while casting to bf16
            rdbc = PS([128, 128], "tr", 2)
            nc.tensor.matmul(
                out=rdbc[:], lhsT=onesmat_bf[32 * h : 32 * h + 1, :],
                rhs=rdbf[32 * h : 32 * h + 1, :], start=True, stop=True,
                tile_position=(32 * h, 0),
            )
            nc.vector.tensor_tensor(out=zaccb[h][:], in0=zacc[:], in1=rdbc[:], op=OP.mult)

        # ---- P11: output ----
        outps = PS([128, 128], "tr", 2)
        for h in range(H):
            nc.tensor.matmul(
                out=outps[:], lhsT=Wh("Wmo", h), rhs=zaccb[h][:], start=(h == 0), stop=False,
                skip_group_check=True,
            )
            nc.tensor.matmul(
                out=outps[:], lhsT=W["wmo1m"][32 * h : 32 * h + 1, 128 * h : 128 * (h + 1)],
                rhs=s3rb[32 * h : 32 * h + 1, :], start=False, stop=(h == H - 1),
                tile_position=(32 * h, 0), skip_group_check=True,
            )
        outsb = tl.tile([128, 128], F32, tag="outsb")
        nc.scalar.activation(out=outsb[:], in_=outps[:], func=AF.Identity, bias=W["bmo"][:])
        trp = PS([128, 128], "tr", 2)
        nc.tensor.transpose(out=trp[:], in_=outsb[:], identity=ident[:])
        outrm = tl.tile([128, 128], F32, tag="outrm")
        nc.vector.tensor_copy(out=outrm[:], in_=trp[:])
        nc.sync.dma_start(out=out_d[slice(128 * t, 128 * (t + 1)), :], in_=outrm[:])

    # software pipeline: gathers/scores run ahead of bodies
    heads = {}
    heads[0] = emit_head(0)
    heads[1] = emit_head(1)
    for t in range(QT):
        if t + 2 < QT:
            pass
        emit_body(t, heads[t])
        del heads[t]
        if t + 2 < QT:
            heads[t + 2] = emit_head(t + 2)

    for p in reversed(_pools):
        p.release()


# ======================= host side =======================


def _host_prep(inputs):
    f = {k: np.asarray(v, np.float32) for k, v in inputs.items()}

    def bf(x):
        return np.ascontiguousarray(np.asarray(x, np.float32)).astype(ml_dtypes.bfloat16)

    def col(x):
        return np.ascontiguousarray(np.asarray(x, np.float32).reshape(-1, 1))

    rff = np.concatenate([FQ * f["rffq"], FV * f["rffv"]], axis=1)  # [2,128]

    wq_s = f["wq"] * SCALE
    bq_s = f["bq"] * SCALE
    W_qm = f["eq_w2"] @ wq_s
    b_qm = f["eq_b2"] @ wq_s + bq_s
    Mq = np.zeros((128, 512), np.float32)
    w1v = np.zeros((1, 512), np.float32)
    w2v = np.zeros((128, 4), np.float32)
    attconst = np.zeros((128, 1), np.float32)
    for h in range(H):
        sl = slice(128 * h, 128 * (h + 1))
        Wq_h = W_qm[:, sl]
        wk_h = f["wk"][:, sl]
        bk_h = f["bk"][sl]
        bq_h = b_qm[sl]
        Mq[:, sl] = Wq_h @ wk_h.T
        w1v[0, sl] = wk_h @ bq_h
        w2v[:, h] = Wq_h @ bk_h
        attconst[32 * h, 0] = float(bq_h @ bk_h)

    # split eq_w1 / ev_w1 into sin/cos input halves
    ws1 = np.zeros((128, 128), np.float32)
    wc1 = np.zeros((128, 128), np.float32)
    ws1[0:64, :] = f["eq_w1"][0:64, :]
    wc1[0:64, :] = f["eq_w1"][64:128, :]
    ws1[64:128, :] = f["ev_w1"][0:64, :]
    wc1[64:128, :] = f["ev_w1"][64:128, :]

    ivw2f = f["ivls"][:, None] * f["ivw2"]
    ivb2f = f["ivb2"] + f["ivlb"] @ f["ivw2"]
    ivw2g = ivw2f[:, :HD]
    ivw2b = ivw2f[:, HD:]
    # bilinear expansion: m1 = mw1.T (v0*vg) + WA.T cg + WB.T ziv + mb1p
    WA = np.zeros((128, 512), np.float32)
    WB = np.zeros((128, 512), np.float32)
    mb1p = np.zeros((128, H), np.float32)
    for h in range(H):
        sl = slice(128 * h, 128 * (h + 1))
        c1_h = 1.0 + ivb2f[:HD][sl]
        bv_h = f["bv"][sl]
        b2_h = ivb2f[HD:][sl]
        WA[:, sl] = f["wv"][:, sl] @ np.diag(c1_h) @ f["mw1"]
        WB[:, sl] = (ivw2g[:, sl] @ np.diag(bv_h) + ivw2b[:, sl]) @ f["mw1"]
        mb1p[:, h] = f["mb1"] + (bv_h * c1_h + b2_h) @ f["mw1"]

    mw2f = f["mls"][:, None] * f["mw2"]
    mb2f = f["mb2"] + f["mlb"] @ f["mw2"]
    Wmo = np.zeros((128, 512), np.float32)
    wmo1m = np.zeros((128, 512), np.float32)
    for h in range(H):
        wo_h = f["wo"][128 * h : 128 * (h + 1), :]
        Wmo_h = mw2f @ wo_h
        Wmo[:, 128 * h : 128 * (h + 1)] = Wmo_h
        wmo1m[32 * h, 128 * h : 128 * (h + 1)] = -Wmo_h.sum(axis=0)
    bmo = f["bo"] + sum(mb2f @ f["wo"][128 * h : 128 * (h + 1), :] for h in range(H))

    cw2f = f["cls"][:, None] * f["cw2"]
    cb2f = f["cb2"] + f["clb"] @ f["cw2"]

    weights = {
        "rff": np.ascontiguousarray(rff),
        "ws1": bf(ws1),
        "wc1": bf(wc1),
        "eq_b1": col(f["eq_b1"]),
        "Mq": bf(Mq),
        "w1v": bf(w1v),
        "w2v": bf(w2v),
        "attconst": attconst.astype(np.float32),
        "ev_b1": col(f["ev_b1"]),
        "ev_w2": bf(f["ev_w2"]),
        "ev_b2": col(f["ev_b2"]),
        "ivw1": bf(f["ivw1"]),
        "ivb1": col(f["ivb1"]),
        "ivw2g": bf(ivw2g),
        "wv": bf(f["wv"]),
        "WA": bf(WA),
        "WB": bf(WB),
        "mw1": bf(f["mw1"]),
        "mb1p": np.ascontiguousarray(mb1p),
        "Wmo": bf(Wmo),
        "wmo1m": bf(wmo1m),
        "bmo": col(bmo),
        "cw1": bf(f["cw1"]),
        "cb1": col(f["cb1"]),
        "cw2g": bf(cw2f[:, :DH]),
        "cw2b": bf(cw2f[:, DH:]),
        "cb2g1": col(cb2f[:DH] + 1.0),
        "cb2b": col(cb2f[DH:]),
    }

    x_flat = f["x"].reshape(B * N, CD)
    xh_flat = f["x_h"].reshape(B * N, DH)

    in_maps = []
    for i in range(NCORES):
        b = (i * NQ) // N
        rs = slice(i * NQ, (i + 1) * NQ)
        p_b = f["p"][b]
        c_b = f["c"][b]
        sig_b = f["window_sigma"][b]
        inv2 = (1.0 / (sig_b[:, 0] ** 2)).astype(np.float16)
        pb = (p_b @ rff).astype(np.float32)
        pbf = (pb - np.rint(pb)).astype(np.float16)
        gtbl = np.zeros((L, 3 * D), ml_dtypes.bfloat16)
        gtbl[:, :D] = bf(c_b)
        gtbl[:, D : 2 * D] = pbf.view(ml_dtypes.bfloat16)
        gtbl[:, 2 * D] = inv2.view(ml_dtypes.bfloat16)
        m = {
            "x": np.ascontiguousarray(x_flat[rs]),
            "xh": np.ascontiguousarray(xh_flat[rs]),
            "gtbl": gtbl,
            "p2t": np.ascontiguousarray((2.0 * p_b).T),
            "npsq": np.ascontiguousarray(-(p_b**2).sum(1)[None, :]),
        }
        m.update(weights)
        in_maps.append(m)
    return in_maps


_PROGRAM_CACHE = {}


def kernel(**inputs):
    in_maps = _host_prep(inputs)
    if "nc" not in _PROGRAM_CACHE:
        _PROGRAM_CACHE["nc"] = build_program()
    nc = _PROGRAM_CACHE["nc"]

    from concourse.bass_utils import run_bass_kernel_spmd

    res = run_bass_kernel_spmd(nc, in_maps, core_ids=list(range(NCORES)))
    outs = [np.asarray(res.results[i]["out"], np.float32) for i in range(NCORES)]
    return np.concatenate(outs, axis=0).reshape(B, N, DH)



# revision 19
# speedup vs baseline: 1.0057x; 1.0057x over previous
"""EquivariantCrossAttention Trainium2 kernel (8 NeuronCores, SPMD) — v2.

kernel(**inputs) takes the FULL unsharded inputs from reference's
setup_inputs() and returns the FULL (B, N, DH) float32 output.

Sharding: flattened query axis (B*N = 4096) split into 8 shards of 512
queries; core i gets queries [512*i, 512*(i+1)) plus its batch's latent
tables. Weights replicated.

Hardcoded problem shapes: B=2 N=2048 L=1024 K=16 CD=2 H=4 DH=128 HD=512.

v2 was 749.6us. v3 changes:
  - cross-tile ACT-table phase pipeline (4 set loads/tile instead of ~9.5):
    [silu: sins(t)+gms(t-1)] [exp: att_e(t-1)] [gelu_tanh: g2/ev/iv1(t)]
    [absrsqrt: iv-rstd(t)+m-rstd(t-1)]; mFFN gelu switched to the sigmoid
    approx via Silu (sin+silu share one table set; 1.702 cancels in LN)
  - small-matmul bursts (LN stats, att reduces, broadcasts) emitted
    back-to-back on distinct row/col groups for PE tile concurrency
  - frac via add_range_wrap custom DVE op (kills int-cast + abs chain);
    cos arg = wrap(tfull + 0.25), so both sins share scale with no bias
  - gsq/vg evacuations moved to ScalarE (Square/Copy are in every set)
"""

import sys

sys.path.insert(0, "/opt/trn_rl_repo")

import numpy as np
import ml_dtypes

import concourse.bass as bass
import concourse.bacc as bacc
import concourse.mybir as mybir
import concourse.tile as tile
from concourse.masks import make_identity

F32 = mybir.dt.float32
BF16 = mybir.dt.bfloat16
F16 = mybir.dt.float16
U32 = mybir.dt.uint32
I32 = mybir.dt.int32
I16 = mybir.dt.int16
AF = mybir.ActivationFunctionType
OP = mybir.AluOpType
AX = mybir.AxisListType

B, N, L, K, CD, H, DH, D = 2, 2048, 1024, 16, 2, 4, 128, 128
HD = H * DH
FQ = 2.0
FV = 2.0
SCALE = 1.0 / float(np.sqrt(DH))
NCORES = 8
NQ = (B * N) // NCORES  # queries per core = 512
QT = NQ // 128  # query tiles per core = 4
NCH = 4  # chunks per query tile
CQ = 128 // NCH  # queries per chunk = 32
CR = CQ * K  # rows per chunk = 512
TR = 128 * K  # rows per tile = 2048
GELU = AF.Gelu_apprx_tanh
TWO_PI = 2.0 * np.pi

WSPECS = [
    ("rff", [CD, 128], F32),
    ("ws1", [128, 128], BF16),   # rows 0:64 eq_w1 sin-half, 64:128 ev_w1 sin-half
    ("wc1", [128, 128], BF16),   # rows 0:64 eq_w1 cos-half, 64:128 ev_w1 cos-half
    ("eq_b1", [128, 1], F32),
    ("Mq", [128, 512], BF16),
    ("w1v", [1, 512], BF16),
    ("w2v", [128, 4], BF16),
    ("attconst", [128, 1], F32),
    ("ev_b1", [128, 1], F32),
    ("ev_w2", [128, 128], BF16),
    ("ev_b2", [128, 1], F32),
    ("ivw1", [128, 128], BF16),
    ("ivb1", [128, 1], F32),
    ("ivw2g", [128, 512], BF16),
    ("wv", [128, 512], BF16),
    ("WA", [128, 512], BF16),
    ("WB", [128, 512], BF16),
    ("mw1", [128, 128], BF16),
    ("mb1p", [128, 4], F32),
    ("Wmo", [128, 512], BF16),
    ("wmo1m", [128, 512], BF16),  # row 32h, cols 128h:128h+128 = -(Wmo_h^T @ 1)
    ("bmo", [128, 1], F32),
    ("cw1", [128, 128], BF16),
    ("cb1", [128, 1], F32),
    ("cw2g", [128, 128], BF16),
    ("cw2b", [128, 128], BF16),
    ("cb2g1", [128, 1], F32),
    ("cb2b", [128, 1], F32),
]


def _bcast_inner(ap, n):
    """[.., Q] AP -> [.., Q, n] with a stride-0 inner dim (free broadcast)."""
    newap = [list(p) for p in ap.ap] + [[0, n]]
    return bass.AP(ap.tensor, ap.offset, newap)


def build_program():
    nc = bacc.Bacc()

    x_d = nc.declare_dram_parameter("x", [NQ, CD], F32, isOutput=False)
    xh_d = nc.declare_dram_parameter("xh", [NQ, DH], F32, isOutput=False)
    gtbl_d = nc.declare_dram_parameter("gtbl", [L, 3 * D], BF16, isOutput=False)
    p2t_d = nc.declare_dram_parameter("p2t", [CD, L], F32, isOutput=False)
    npsq_d = nc.declare_dram_parameter("npsq", [1, L], F32, isOutput=False)
    w_d = {}
    for name, shape, dt in WSPECS:
        w_d[name] = nc.declare_dram_parameter(name, shape, dt, isOutput=False)
    out_d = nc.declare_dram_parameter("out", [NQ, DH], F32, isOutput=True)

    with tile.TileContext(nc) as tc:
        _emit(nc, tc, x_d, xh_d, gtbl_d, p2t_d, npsq_d, w_d, out_d)
    nc.compile()
    return nc


def _emit(nc, tc, x_d, xh_d, gtbl_d, p2t_d, npsq_d, w_d, out_d):
    const = tc.alloc_tile_pool(name="const", bufs=1)
    wpool = tc.alloc_tile_pool(name="wpool", bufs=1)
    core = tc.alloc_tile_pool(name="core", bufs=1)
    hd = tc.alloc_tile_pool(name="hd", bufs=1)   # per-tile head tiles
    tl = tc.alloc_tile_pool(name="tl", bufs=1)   # per-tile body tiles
    ck = tc.alloc_tile_pool(name="ck", bufs=1)   # per-chunk transient tiles
    psp = tc.alloc_tile_pool(name="psp", bufs=1, space="PSUM")
    _pools = [const, wpool, core, hd, tl, ck, psp]

    _psn = [0]

    def PS(shape, tag, bufs, dtype=F32):
        _psn[0] += 1
        return psp.tile(
            shape, dtype, space="PSUM", tag=tag, bufs=bufs, name=f"ps_{tag}_{_psn[0]}"
        )

    # ---------- constants ----------
    ident = const.tile([128, 128], F32)
    make_identity(nc, ident[:])

    ones_col_bf = const.tile([128, 1], BF16)
    nc.vector.memset(ones_col_bf[:], 1.0)
    inv128_bf = const.tile([128, 1], BF16)
    nc.vector.memset(inv128_bf[:], 1.0 / 128.0)
    ones_row_bf = const.tile([1, 128], BF16)
    nc.vector.memset(ones_row_bf[:], 1.0)
    ones_row512_bf = const.tile([1, 512], BF16)
    nc.vector.memset(ones_row512_bf[:], 1.0)
    half_row128_bf = const.tile([1, 128], BF16)
    nc.vector.memset(half_row128_bf[:], 0.5)
    onesmat_bf = const.tile([128, 128], BF16)
    nc.vector.memset(onesmat_bf[:], 1.0)
    zeros_row_bf = const.tile([1, 128], BF16)
    nc.vector.memset(zeros_row_bf[:], 0.0)
    zeros_row512_bf = const.tile([1, 512], BF16)
    nc.vector.memset(zeros_row512_bf[:], 0.0)
    ones_row_f32 = const.tile([1, 128], F32)
    nc.vector.memset(ones_row_f32[:], 1.0)
    pihalf = const.tile([128, 1], F32)
    nc.vector.memset(pihalf[:], np.pi / 2.0)
    eps_col = const.tile([128, 1], F32)
    nc.vector.memset(eps_col[:], 1e-6)

    # ---------- weights ----------
    W = {}
    for name, shape, dt in WSPECS:
        wt = wpool.tile(shape, dt, name=f"w_{name}", tag=f"w_{name}")
        nc.sync.dma_start(out=wt[:], in_=w_d[name][:])
        W[name] = wt

    def Wh(name, h, w=128):
        return W[name][:, h * w : (h + 1) * w]

    # ---------- per-core precompute ----------
    x_sb = core.tile([128, QT, CD], F32)
    nc.sync.dma_start(out=x_sb[:], in_=x_d[:].rearrange("(t q) c -> q t c", q=128))
    xsq = core.tile([128, QT], F32)
    xs2 = core.tile([128, QT, CD], F32)
    nc.vector.tensor_tensor(out=xs2[:], in0=x_sb[:], in1=x_sb[:], op=OP.mult)
    nc.vector.tensor_reduce(out=xsq[:], in_=xs2[:], axis=AX.X, op=OP.add)

    x_fm = core.tile([CD, NQ], F32)
    for t in range(QT):
        tp = PS([CD, 128], "tr", 2)
        nc.tensor.transpose(out=tp[:], in_=x_sb[:, t, :], identity=ident[:])
        nc.vector.tensor_copy(out=x_fm[:, 128 * t : 128 * (t + 1)], in_=tp[:])

    p2_fm = core.tile([CD, L], F32)
    nc.sync.dma_start(out=p2_fm[:], in_=p2t_d[:])
    npsq = core.tile([1, L], F32)
    nc.sync.dma_start(out=npsq[:], in_=npsq_d[:])

    # ---------- cFFN on x_h (512 queries at once) ----------
    xh_fm = core.tile([128, NQ], BF16)
    xh_rm = core.tile([128, QT, DH], F32)
    nc.sync.dma_start(out=xh_rm[:], in_=xh_d[:].rearrange("(t q) c -> q t c", q=128))
    for t in range(QT):
        tp = PS([128, 128], "tr", 2)
        nc.tensor.transpose(out=tp[:], in_=xh_rm[:, t, :], identity=ident[:])
        nc.vector.tensor_copy(out=xh_fm[:, 128 * t : 128 * (t + 1)], in_=tp[:])

    c1ps = PS([128, NQ], "mm", 4)
    nc.tensor.matmul(out=c1ps[:], lhsT=W["cw1"][:], rhs=xh_fm[:], start=True, stop=True)
    c1 = ck.tile([128, NQ], BF16, tag="ev1", bufs=2)
    nc.scalar.activation(out=c1[:], in_=c1ps[:], func=GELU, bias=W["cb1"][:])

    cmean = PS([128, NQ], "stat", 2)
    nc.tensor.matmul(out=cmean[0:1, :], lhsT=inv128_bf[:], rhs=c1[:], start=True, stop=True)
    c1sq = ck.tile([128, NQ], BF16, tag="vg_sb", bufs=1)
    nc.vector.tensor_tensor(out=c1sq[:], in0=c1[:], in1=c1[:], op=OP.mult)
    cmsq = PS([128, NQ], "stat", 2)
    nc.tensor.matmul(out=cmsq[0:1, :], lhsT=inv128_bf[:], rhs=c1sq[:], start=True, stop=True)

    cm2 = core.tile([1, NQ], F32)
    nc.scalar.square(out=cm2[:], in_=cmean[0:1, :])
    cvar = core.tile([1, NQ], F32)
    nc.vector.tensor_tensor(out=cvar[:], in0=cmsq[0:1, :], in1=cm2[:], op=OP.subtract)
    crstd_bf = core.tile([1, NQ], BF16)
    nc.scalar.activation(
        out=crstd_bf[:], in_=cvar[:], func=AF.Abs_reciprocal_sqrt, bias=eps_col[0:1, :]
    )
    cmr_bf = core.tile([1, NQ], BF16)
    nc.vector.tensor_tensor(out=cmr_bf[:], in0=cmean[0:1, :], in1=crstd_bf[:], op=OP.mult)
    crbc = PS([128, NQ], "mm", 4)
    nc.tensor.matmul(out=crbc[:], lhsT=ones_row_bf[:], rhs=crstd_bf[:], start=True, stop=True)
    cmbc = PS([128, NQ], "mm", 4)
    nc.tensor.matmul(out=cmbc[:], lhsT=ones_row_bf[:], rhs=cmr_bf[:], start=True, stop=True)
    z1c = ck.tile([128, NQ], BF16, tag="z1", bufs=1)
    nc.vector.tensor_tensor(out=z1c[:], in0=c1[:], in1=crbc[:], op=OP.mult)
    zc = ck.tile([128, NQ], BF16, tag="p_sb", bufs=2)
    nc.vector.tensor_tensor(out=zc[:], in0=z1c[:], in1=cmbc[:], op=OP.subtract)

    gp1 = core.tile([128, NQ], BF16)
    modadd = core.tile([128, NQ], BF16)
    gps_ = PS([128, NQ], "mm", 4)
    nc.tensor.matmul(out=gps_[:], lhsT=W["cw2g"][:], rhs=zc[:], start=True, stop=True)
    nc.scalar.activation(out=gp1[:], in_=gps_[:], func=AF.Identity, bias=W["cb2g1"][:])
    btps = PS([128, NQ], "mm", 4)
    nc.tensor.matmul(out=btps[:], lhsT=W["cw2b"][:], rhs=zc[:], start=True, stop=True)
    bt = ck.tile([128, NQ], BF16, tag="ivsq", bufs=2)
    nc.scalar.activation(out=bt[:], in_=btps[:], func=AF.Identity, bias=W["cb2b"][:])
    ma1 = ck.tile([128, NQ], BF16, tag="zp", bufs=2)
    nc.vector.tensor_scalar(
        out=ma1[:], in0=gp1[:], scalar1=W["ev_b2"][:], scalar2=None, op0=OP.mult
    )
    nc.vector.tensor_tensor(out=modadd[:], in0=ma1[:], in1=bt[:], op=OP.add)

    # ================= per query tile: head phase =================
    # scores -> top-16 -> idx16 -> combined gather -> t_x frac
    def emit_head(t):
        qs = slice(128 * t, 128 * (t + 1))

        scores = hd.tile([128, L], F32, tag="scores", bufs=1, name=f"scores_{t}")
        for s in range(2):
            sl = slice(512 * s, 512 * (s + 1))
            scps = PS([128, 512], "mm", 4)
            nc.tensor.matmul(out=scps[:], lhsT=x_fm[:, qs], rhs=p2_fm[:, sl], start=True, stop=False)
            nc.tensor.matmul(out=scps[:], lhsT=ones_row_f32[:], rhs=npsq[:, sl], start=False, stop=True)
            nc.scalar.copy(out=scores[:, sl], in_=scps[:])
        vals = hd.tile([128, K], F32, tag="vals", bufs=2, name=f"vals_{t}")
        idxs = hd.tile([128, K], U32, tag="idxs", bufs=2, name=f"idxs_{t}")
        scr2 = hd.tile([128, L], F32, tag="scr2", bufs=1, name=f"scr2_{t}")
        nc.vector.max(out=vals[:, 0:8], in_=scores[:])
        nc.vector.max_index(out=idxs[:, 0:8], in_max=vals[:, 0:8], in_values=scores[:])
        nc.vector.match_replace(
            out=scr2[:], in_to_replace=vals[:, 0:8], in_values=scores[:], imm_value=-1e30
        )
        nc.vector.max(out=vals[:, 8:16], in_=scr2[:])
        nc.vector.max_index(out=idxs[:, 8:16], in_max=vals[:, 8:16], in_values=scr2[:])

        # index prep for dma_gather: idx16[k, q] (int16) replicated across
        # the 8 gpsimd cores' 16-partition blocks
        idxf = hd.tile([128, K], F32, tag="idxf", bufs=1, name=f"idxf_{t}")
        nc.vector.tensor_copy(out=idxf[:], in_=idxs[:])
        idxt_ps = PS([K, 128], "tr", 2)
        nc.tensor.transpose(out=idxt_ps[:], in_=idxf[:], identity=ident[:])
        idx16 = hd.tile([128, 128], I16, tag="idx16", bufs=2, name=f"idx16_{t}")
        nc.vector.tensor_copy(out=idx16[0:16, :], in_=idxt_ps[:])
        for b in range(1, 8):
            nc.sync.dma_start(out=idx16[16 * b : 16 * b + 16, :], in_=idx16[0:16, :])

        # ONE combined transposed gather for the whole tile (2048 rows):
        # G[128, 0, :] = c (bf16), G[128, 1, :] = frac(p@Bs) (f16 bits),
        # G[0, 2, :] = 1/sigma^2 (f16 bits)
        Gs = []
        for c in range(NCH):
            G = hd.tile([128, 3, CR], BF16, tag=f"G{c}", bufs=2, name=f"G{c}_{t}")
            nc.gpsimd.dma_gather(
                out_ap=G[:], in_ap=gtbl_d[:],
                idxs_ap=idx16[:, 32 * c : 32 * c + 32], num_idxs=CR,
                num_idxs_reg=CR, elem_size=3 * D, transpose=True,
            )
            Gs.append(G)

        # -d^2 row [1, 2048] (q-major) via sbuf->sbuf DMA flatten
        negd2 = hd.tile([128, K], BF16, tag="negd2", bufs=2, name=f"negd2_{t}")
        nc.vector.tensor_scalar(
            out=negd2[:], in0=vals[:], scalar1=xsq[:, t : t + 1], scalar2=None,
            op0=OP.subtract,
        )
        negd2_row = hd.tile([1, TR], BF16, tag="negd2_row", bufs=2, name=f"nd2r_{t}")
        nc.sync.dma_start(out=negd2_row[:], in_=negd2[:])

        # t_x = x @ Bs for this tile, pre-frac'd: txf = t_x - rint(t_x), f16
        txps = PS([128, 128], "tr", 2)
        nc.tensor.matmul(out=txps[:], lhsT=W["rff"][:], rhs=x_fm[:, qs], start=True, stop=True)
        t_x = hd.tile([128, 128], F32, tag="t_x", bufs=1, name=f"t_x_{t}")
        nc.vector.tensor_copy(out=t_x[:], in_=txps[:])
        txi = hd.tile([128, 128], I32, tag="txi", bufs=1, name=f"txi_{t}")
        nc.vector.tensor_copy(out=txi[:], in_=t_x[:])
        txf = hd.tile([128, 128], F32, tag="txf", bufs=2, name=f"txf_{t}")
        nc.vector.tensor_tensor(out=txf[:], in0=t_x[:], in1=txi[:], op=OP.subtract)
        return dict(Gs=Gs, negd2_row=negd2_row, txf=txf)

    # ================= per query tile: phased body =================
    # ACT table-set phases per steady-state iteration t:
    #   [silu]      sins(t) + mFFN gelus gms(t-1)     (Sin+Silu share a set)
    #   [gelu_tanh] g2/ev1/iv1 gelus (t) + att logits
    #   [absrsqrt]  iv-rstd(t) + m-rstd(t-1)
    #   [exp]       softmax att_e(t-1), then P10/P11(t-1) (no ACT sets)
    # Copy/Identity/Square/Abs are in every set (free).

    state = {}

    def emit_sins(t, ht):
        Gs = ht["Gs"]
        negd2_row = ht["negd2_row"]
        txf = ht["txf"]

        # ---- RFF frac chain (DVE) ----
        # tfull = txf - pbf in [-1, 1]; range-wrap replaces the int-cast
        # round: fs = wrap(tfull) in [-.5, .5]; fc = wrap(tfull + .25) so
        # sin(2*pi*fc) = cos(2*pi*tfull) with no abs / bias needed.
        tfull = tl.tile([128, TR], F16, tag="tfull", bufs=1, name=f"tfull_{t}")
        for c in range(NCH):
            cs = slice(CR * c, CR * (c + 1))
            nc.vector.tensor_tensor(
                out=tfull[:, cs].rearrange("p (a b) -> p a b", a=CQ),
                in0=_bcast_inner(txf[:, CQ * c : CQ * (c + 1)], K),
                in1=Gs[c][:, 1, :].bitcast(F16).rearrange("p (a b) -> p a b", a=CQ),
                op=OP.subtract,
            )
        fs = tl.tile([128, TR], F16, tag="fs", bufs=1, name=f"fs_{t}")
        nc.vector.add_range_wrap(out=fs[:], in_=tfull[:], shift=0.0, bound=0.5, period=1.0)
        fc = tl.tile([128, TR], F16, tag="fc", bufs=1, name=f"fc_{t}")
        nc.vector.add_range_wrap(out=fc[:], in_=tfull[:], shift=0.25, bound=0.5, period=1.0)
        # gw row [1, 2048] = -d^2 * (1/sigma^2)
        gw_row = tl.tile([1, TR], BF16, tag="gw_row", bufs=1, name=f"gw_{t}")
        for c in range(NCH):
            cs = slice(CR * c, CR * (c + 1))
            nc.vector.tensor_tensor(
                out=gw_row[:, cs], in0=negd2_row[:, cs],
                in1=Gs[c][0:1, 2, :].bitcast(F16), op=OP.mult,
            )

        # ---- sins (silu set) ----
        sfull = tl.tile([128, TR], BF16, tag="sfull", bufs=1, name=f"sfull_{t}")
        cfull = tl.tile([128, TR], BF16, tag="cfull", bufs=1, name=f"cfull_{t}")
        nc.scalar.activation(out=sfull[:], in_=fs[:], func=AF.Sin, scale=TWO_PI)
        nc.scalar.activation(out=cfull[:], in_=fc[:], func=AF.Sin, scale=TWO_PI)
        st = state.setdefault(t, {})
        st.update(sfull=sfull, cfull=cfull, gw_row=gw_row, Gs=Gs)

    def emit_geluA(t):
        st = state[t]
        sfull, cfull, gw_row, Gs = st["sfull"], st["cfull"], st["gw_row"], st["Gs"]

        def cg(c):
            return Gs[c][:, 0, :]

        # ---- q path + attention logits ----
        g2 = tl.tile([128, TR], BF16, tag="g2", bufs=1, name=f"g2_{t}")
        att_l = tl.tile([128, TR], BF16, tag="att_l", bufs=1, name=f"attl_{t}")
        iv1 = tl.tile([128, TR], BF16, tag="iv1", bufs=1, name=f"iv1_{t}")
        ivmean_row = tl.tile([1, TR], BF16, tag="ivmean_row", bufs=1, name=f"ivmr_{t}")
        ivar_all = tl.tile([1, TR], BF16, tag="ivar_all", bufs=1, name=f"ivva_{t}")
        for c in range(NCH):
            cs = slice(CR * c, CR * (c + 1))
            gq = slice(128 * t + CQ * c, 128 * t + CQ * (c + 1))
            # g2 (rows 0:64) and ev1 (rows 64:128) interleaved: disjoint
            # row-groups let the PE overlap the pairs
            g2ps = PS([128, CR], "mm", 4)
            ev1ps = PS([128, CR], "mm", 4)
            nc.tensor.matmul(out=g2ps[:], lhsT=W["ws1"][0:64, :], rhs=sfull[0:64, cs], start=True, stop=False)
            nc.tensor.matmul(out=ev1ps[:], lhsT=W["ws1"][64:128, :], rhs=sfull[64:128, cs], start=True, stop=False)
            nc.tensor.matmul(out=g2ps[:], lhsT=W["wc1"][0:64, :], rhs=cfull[0:64, cs], start=False, stop=True)
            nc.tensor.matmul(out=ev1ps[:], lhsT=W["wc1"][64:128, :], rhs=cfull[64:128, cs], start=False, stop=True)
            nc.scalar.activation(out=g2[:, cs], in_=g2ps[:], func=GELU, bias=W["eq_b1"][:])
            ev1 = ck.tile([128, CR], BF16, tag="ev1", bufs=2)
            nc.scalar.activation(out=ev1[:], in_=ev1ps[:], func=GELU, bias=W["ev_b1"][:])

            # attention: 4 full MMs (+qkp mults), then the 8 small reduce
            # MMs back-to-back on distinct col-groups (concurrent on PE)
            attps = PS([128, CR], "stat", 2)
            nc.tensor.matmul(
                out=attps[:], lhsT=half_row128_bf[:],
                rhs=gw_row[:, cs], start=True, stop=False,
                skip_group_check=True,
            )
            qkp = ck.tile([128, H, CR], BF16, tag="qkp", bufs=1)
            for h in range(H):
                ups = PS([128, CR], "mm", 4)
                nc.tensor.matmul(out=ups[:], lhsT=Wh("Mq", h), rhs=g2[:, cs], start=True, stop=False)
                nc.tensor.matmul(
                    out=ups[:], lhsT=W["w1v"][:, 128 * h : 128 * (h + 1)],
                    rhs=ones_row512_bf[:], start=False, stop=True,
                )
                nc.vector.tensor_tensor(out=qkp[:, h, :], in0=ups[:], in1=cg(c), op=OP.mult)
            for h in range(H):
                nc.tensor.matmul(
                    out=attps[32 * h : 32 * h + 1, :], lhsT=ones_col_bf[:], rhs=qkp[:, h, :],
                    start=False, stop=False, tile_position=(0, 32 * h),
                    skip_group_check=True,
                )
            for h in range(H):
                nc.tensor.matmul(
                    out=attps[32 * h : 32 * h + 1, :], lhsT=W["w2v"][:, h : h + 1],
                    rhs=g2[:, cs], start=False, stop=(h == H - 1), tile_position=(0, 32 * h),
                    skip_group_check=True,
                )
            nc.scalar.copy(out=att_l[:, cs], in_=attps[:])

            # ---- ev modulation -> iv1 ----
            ev2ps = PS([128, CR], "mm", 4)
            nc.tensor.matmul(out=ev2ps[:], lhsT=W["ev_w2"][:], rhs=ev1[:], start=True, stop=True)
            mv = ck.tile([128, CQ, K], BF16, tag="mv", bufs=1)
            nc.vector.tensor_tensor(
                out=mv[:], in0=ev2ps[:].rearrange("p (a b) -> p a b", a=CQ),
                in1=_bcast_inner(gp1[:, gq], K), op=OP.mult,
            )
            ivin = ck.tile([128, CQ, K], BF16, tag="ivin", bufs=2)
            nc.vector.tensor_tensor(
                out=ivin[:], in0=mv[:], in1=_bcast_inner(modadd[:, gq], K), op=OP.add
            )
            iv1ps = PS([128, CR], "mm", 4)
            nc.tensor.matmul(
                out=iv1ps[:], lhsT=W["ivw1"][:],
                rhs=ivin[:].rearrange("p a b -> p (a b)"), start=True, stop=True,
            )
            nc.scalar.activation(out=iv1[:, cs], in_=iv1ps[:], func=GELU, bias=W["ivb1"][:])

            # ---- iv LN stats (PE + in-set Square/Copy); rstd deferred ----
            ivst = PS([128, CR], "stat", 2)
            ivsq = ck.tile([128, CR], BF16, tag="ivsq", bufs=2)
            nc.vector.tensor_tensor(out=ivsq[:], in0=iv1[:, cs], in1=iv1[:, cs], op=OP.mult)
            nc.tensor.matmul(
                out=ivst[0:1, :], lhsT=inv128_bf[:], rhs=iv1[:, cs],
                start=True, stop=True, skip_group_check=True,
            )
            nc.tensor.matmul(
                out=ivst[32:33, :], lhsT=inv128_bf[:], rhs=ivsq[:],
                start=True, stop=True, tile_position=(0, 32),
                skip_group_check=True,
            )
            nc.scalar.copy(out=ivmean_row[:, cs], in_=ivst[0:1, :])
            im2 = ck.tile([1, CR], BF16, tag="im2")
            nc.scalar.square(out=im2[:], in_=ivst[0:1, :])
            nc.vector.tensor_tensor(
                out=ivar_all[:, cs], in0=ivst[32:33, :], in1=im2[:], op=OP.subtract
            )
        st.update(g2=g2, att_l=att_l, iv1=iv1, ivmean_row=ivmean_row, ivar_all=ivar_all)

    def emit_ivrstd(t):
        # [absrsqrt set] one rstd for the whole tile, then LN apply (DVE/PE)
        st = state[t]
        iv1, ivmean_row, ivar_all = st["iv1"], st["ivmean_row"], st["ivar_all"]
        ivs33 = tl.tile([33, TR], BF16, tag="ivs33", bufs=1, name=f"ivs33_{t}")
        nc.scalar.activation(
            out=ivs33[0:1, :], in_=ivar_all[:], func=AF.Abs_reciprocal_sqrt,
            bias=eps_col[0:1, :],
        )
        # imr at partition 32 so its broadcast MM uses row-group 1 (overlaps
        # with irbc's row-group 0)
        nc.vector.tensor_tensor(
            out=ivs33[32:33, :], in0=ivmean_row[:], in1=ivs33[0:1, :], op=OP.mult
        )
        ziv = tl.tile([128, TR], BF16, tag="ziv", bufs=1, name=f"ziv_{t}")
        for c in range(NCH):
            cs = slice(CR * c, CR * (c + 1))
            irbc = PS([128, CR], "mm", 4)
            imbc = PS([128, CR], "mm", 4)
            nc.tensor.matmul(out=irbc[:], lhsT=ones_row_bf[:], rhs=ivs33[0:1, cs], start=True, stop=True)
            nc.tensor.matmul(
                out=imbc[:], lhsT=onesmat_bf[32:33, :], rhs=ivs33[32:33, cs],
                start=True, stop=True, tile_position=(32, 0),
            )
            z1 = ck.tile([128, CR], BF16, tag="z1", bufs=1)
            nc.vector.tensor_tensor(out=z1[:], in0=iv1[:, cs], in1=irbc[:], op=OP.mult)
            nc.vector.tensor_tensor(out=ziv[:, cs], in0=z1[:], in1=imbc[:], op=OP.subtract)
        st["ziv"] = ziv

    def emit_gms(t):
        # [silu set] mFFN: gelu via silu(1.702x) (=1.702*gelu_sig(x); the
        # 1.702 factor cancels exactly through the LN rstd)
        st = state[t]
        ziv, Gs = st["ziv"], st["Gs"]

        def cg(c):
            return Gs[c][:, 0, :]

        gms = [
            tl.tile([128, TR], BF16, tag=f"gm{h}", bufs=1, name=f"gm{h}_{t}")
            for h in range(H)
        ]
        mvar_all = tl.tile([128, TR], BF16, tag="mvar", bufs=1, name=f"mvar_{t}")
        mmean_sb = tl.tile([128, TR], BF16, tag="mmean_sb", bufs=1, name=f"mmean_{t}")
        for c in range(NCH):
            cs = slice(CR * c, CR * (c + 1))
            mmean = PS([128, CR], "stat", 2)
            msqp = PS([128, CR], "stat", 2)
            gsq = ck.tile([128, H, CR], BF16, tag="gsq", bufs=1)
            for h in range(H):
                vgps = PS([128, CR], "mm", 4)
                nc.tensor.matmul(out=vgps[:], lhsT=Wh("ivw2g", h), rhs=ziv[:, cs], start=True, stop=True)
                vg_sb = ck.tile([128, CR], BF16, tag="vg_sb", bufs=1)
                nc.scalar.copy(out=vg_sb[:], in_=vgps[:])
                v0ps = PS([128, CR], "mm", 4)
                nc.tensor.matmul(out=v0ps[:], lhsT=Wh("wv", h), rhs=cg(c), start=True, stop=True)
                p_sb = ck.tile([128, CR], BF16, tag="p_sb", bufs=2)
                nc.vector.tensor_tensor(out=p_sb[:], in0=v0ps[:], in1=vg_sb[:], op=OP.mult)
                m1ps = PS([128, CR], "mm", 4)
                nc.tensor.matmul(out=m1ps[:], lhsT=W["mw1"][:], rhs=p_sb[:], start=True, stop=False)
                nc.tensor.matmul(out=m1ps[:], lhsT=Wh("WA", h), rhs=cg(c), start=False, stop=False)
                nc.tensor.matmul(out=m1ps[:], lhsT=Wh("WB", h), rhs=ziv[:, cs], start=False, stop=True)
                nc.scalar.activation(
                    out=gms[h][:, cs], in_=m1ps[:], func=AF.Silu, scale=1.702,
                    bias=W["mb1p"][:, h : h + 1],
                )
                nc.scalar.square(out=gsq[:, h, :], in_=gms[h][:, cs])
            # stat MMs in two 4-bursts (col-groups 0..3 back-to-back)
            for h in range(H):
                nc.tensor.matmul(
                    out=mmean[32 * h : 32 * h + 1, :], lhsT=inv128_bf[:], rhs=gms[h][:, cs],
                    start=True, stop=True, tile_position=(0, 32 * h),
                    skip_group_check=True,
                )
            for h in range(H):
                nc.tensor.matmul(
                    out=msqp[32 * h : 32 * h + 1, :], lhsT=inv128_bf[:], rhs=gsq[:, h, :],
                    start=True, stop=True, tile_position=(0, 32 * h),
                    skip_group_check=True,
                )
            mm2 = ck.tile([128, CR], BF16, tag="mm2")
            nc.scalar.square(out=mm2[:], in_=mmean[:])
            nc.vector.tensor_tensor(out=mvar_all[:, cs], in0=msqp[:], in1=mm2[:], op=OP.subtract)
            nc.scalar.copy(out=mmean_sb[:, cs], in_=mmean[:])
        st.update(gms=gms, mvar_all=mvar_all, mmean_sb=mmean_sb)

    def emit_mrstd(t):
        # [absrsqrt set]
        st = state[t]
        mrstd = tl.tile([128, TR], BF16, tag="mrstd", bufs=1, name=f"mrstd_{t}")
        mmr = tl.tile([128, TR], BF16, tag="mmr", bufs=1, name=f"mmr_{t}")
        nc.scalar.activation(
            out=mrstd[:], in_=st["mvar_all"][:], func=AF.Abs_reciprocal_sqrt,
            bias=eps_col[:],
        )
        nc.vector.tensor_tensor(out=mmr[:], in0=st["mmean_sb"][:], in1=mrstd[:], op=OP.mult)
        st.update(mrstd=mrstd, mmr=mmr)

    def emit_exp(t):
        # [exp set] softmax numerator
        st = state[t]
        att_e = tl.tile([128, TR], BF16, tag="att_e", bufs=1, name=f"atte_{t}")
        nc.scalar.activation(out=att_e[:], in_=st["att_l"][:], func=AF.Exp, bias=W["attconst"][:])
        st["att_e"] = att_e

    def emit_out(t):
        # P10/P11 (no new ACT sets: Identity only)
        st = state[t]
        att_e, mrstd, mmr, gms = st["att_e"], st["mrstd"], st["mmr"], st["gms"]
        den_t = tl.tile([128, 128], F32, tag="den_t", name=f"den_{t}")
        nc.vector.tensor_reduce(
            out=den_t[:], in_=att_e[:].rearrange("p (a b) -> p a b", a=128),
            axis=AX.X, op=OP.add,
        )
        rden_t = tl.tile([128, 128], F32, tag="rden_t", name=f"rden_{t}")
        nc.vector.reciprocal(out=rden_t[:], in_=den_t[:])
        rdbf = tl.tile([128, 128], BF16, tag="rdbf", name=f"rdbf_{t}")
        nc.vector.tensor_copy(out=rdbf[:], in_=rden_t[:])

        a2 = tl.tile([128, TR], BF16, tag="a2", name=f"a2_{t}")
        nc.vector.tensor_tensor(out=a2[:], in0=att_e[:], in1=mrstd[:], op=OP.mult)
        a3 = tl.tile([128, TR], BF16, tag="a3", name=f"a3_{t}")
        nc.vector.tensor_tensor(out=a3[:], in0=att_e[:], in1=mmr[:], op=OP.mult)
        s3 = tl.tile([128, 128], F32, tag="s3", name=f"s3_{t}")
        nc.vector.tensor_reduce(
            out=s3[:], in_=a3[:].rearrange("p (a b) -> p a b", a=128), axis=AX.X, op=OP.add
        )
        s3rb = tl.tile([128, 128], BF16, tag="s3rb", name=f"s3rb_{t}")
        nc.vector.tensor_tensor(out=s3rb[:], in0=s3[:], in1=rden_t[:], op=OP.mult)

        zaccb = [
            tl.tile([128, 128], BF16, tag=f"zaccb{h}", name=f"zaccb{h}_{t}", bufs=2)
            for h in range(H)
        ]
        zaccs = [
            tl.tile([128, 128], F32, tag=f"zacc{h}", name=f"zacc{h}_{t}")
            for h in range(H)
        ]
        for c in range(NCH):
            cs = slice(CR * c, CR * (c + 1))
            qsl = slice(CQ * c, CQ * (c + 1))
            # 4 broadcast MMs back-to-back (row-groups 0..3: concurrent)
            a2bcs = []
            for h in range(H):
                a2bc = PS([128, CR], "mm", 4)
                nc.tensor.matmul(
                    out=a2bc[:], lhsT=onesmat_bf[32 * h : 32 * h + 1, :],
                    rhs=a2[32 * h : 32 * h + 1, cs], start=True, stop=True,
                    tile_position=(32 * h, 0),
                )
                a2bcs.append(a2bc)
            for h in range(H):
                zp = ck.tile([128, CR], BF16, tag="zp", bufs=2)
                nc.vector.tensor_tensor(out=zp[:], in0=gms[h][:, cs], in1=a2bcs[h][:], op=OP.mult)
                nc.vector.tensor_reduce(
                    out=zaccs[h][:, qsl], in_=zp[:].rearrange("p (a b) -> p a b", a=CQ),
                    axis=AX.X, op=OP.add,
                )
        for h in range(H):
            rdbc = PS([128, 128], "tr", 2)
            nc.tensor.matmul(
                out=rdbc[:], lhsT=onesmat_bf[32 * h : 32 * h + 1, :],
                rhs=rdbf[32 * h : 32 * h + 1, :], start=True, stop=True,
                tile_position=(32 * h, 0),
            )
            nc.vector.tensor_tensor(out=zaccb[h][:], in0=zaccs[h][:], in1=rdbc[:], op=OP.mult)

        outps = PS([128, 128], "tr", 2)
        for h in range(H):
            nc.tensor.matmul(
                out=outps[:], lhsT=Wh("Wmo", h), rhs=zaccb[h][:], start=(h == 0), stop=False,
                skip_group_check=True,
            )
            nc.tensor.matmul(
                out=outps[:], lhsT=W["wmo1m"][32 * h : 32 * h + 1, 128 * h : 128 * (h + 1)],
                rhs=s3rb[32 * h : 32 * h + 1, :], start=False, stop=(h == H - 1),
                tile_position=(32 * h, 0), skip_group_check=True,
            )
        outsb = tl.tile([128, 128], F32, tag="outsb")
        nc.scalar.activation(out=outsb[:], in_=outps[:], func=AF.Identity, bias=W["bmo"][:])
        trp = PS([128, 128], "tr", 2)
        nc.tensor.transpose(out=trp[:], in_=outsb[:], identity=ident[:])
        outrm = tl.tile([128, 128], F32, tag="outrm")
        nc.vector.tensor_copy(out=outrm[:], in_=trp[:])
        nc.sync.dma_start(out=out_d[slice(128 * t, 128 * (t + 1)), :], in_=outrm[:])
        del state[t]

    # ---- pipelined emission ----
    # iteration t: [silu: sins(t), gms(t-1)] [exp: att_e(t-1)]
    #             [gelu_tanh: geluA(t)] [absrsqrt: ivr(t), mr(t-1)]
    #             then out(t-1) (no ACT sets) + head(t+2)
    heads = {}
    heads[0] = emit_head(0)
    heads[1] = emit_head(1)
    for t in range(QT):
        emit_sins(t, heads[t])
        del heads[t]
        if t - 1 >= 0:
            emit_gms(t - 1)
            emit_exp(t - 1)
        emit_geluA(t)
        emit_ivrstd(t)
        if t - 1 >= 0:
            emit_mrstd(t - 1)
            emit_out(t - 1)
        if t + 2 < QT:
            heads[t + 2] = emit_head(t + 2)
    # tail: tile QT-1
    emit_gms(QT - 1)
    emit_exp(QT - 1)
    emit_mrstd(QT - 1)
    emit_out(QT - 1)

    for p in reversed(_pools):
        p.release()


# ======================= host side =======================


def _host_prep(inputs):
    f = {k: np.asarray(v, np.float32) for k, v in inputs.items()}

    def bf(x):
        return np.ascontiguousarray(np.asarray(x, np.float32)).astype(ml_dtypes.bfloat16)

    def col(x):
        return np.ascontiguousarray(np.asarray(x, np.float32).reshape(-1, 1))

    rff = np.concatenate([FQ * f["rffq"], FV * f["rffv"]], axis=1)  # [2,128]

    wq_s = f["wq"] * SCALE
    bq_s = f["bq"] * SCALE
    W_qm = f["eq_w2"] @ wq_s
    b_qm = f["eq_b2"] @ wq_s + bq_s
    Mq = np.zeros((128, 512), np.float32)
    w1v = np.zeros((1, 512), np.float32)
    w2v = np.zeros((128, 4), np.float32)
    attconst = np.zeros((128, 1), np.float32)
    for h in range(H):
        sl = slice(128 * h, 128 * (h + 1))
        Wq_h = W_qm[:, sl]
        wk_h = f["wk"][:, sl]
        bk_h = f["bk"][sl]
        bq_h = b_qm[sl]
        Mq[:, sl] = Wq_h @ wk_h.T
        w1v[0, sl] = wk_h @ bq_h
        w2v[:, h] = Wq_h @ bk_h
        attconst[32 * h, 0] = float(bq_h @ bk_h)

    # split eq_w1 / ev_w1 into sin/cos input halves
    ws1 = np.zeros((128, 128), np.float32)
    wc1 = np.zeros((128, 128), np.float32)
    ws1[0:64, :] = f["eq_w1"][0:64, :]
    wc1[0:64, :] = f["eq_w1"][64:128, :]
    ws1[64:128, :] = f["ev_w1"][0:64, :]
    wc1[64:128, :] = f["ev_w1"][64:128, :]

    ivw2f = f["ivls"][:, None] * f["ivw2"]
    ivb2f = f["ivb2"] + f["ivlb"] @ f["ivw2"]
    ivw2g = ivw2f[:, :HD]
    ivw2b = ivw2f[:, HD:]
    # bilinear expansion: m1 = mw1.T (v0*vg) + WA.T cg + WB.T ziv + mb1p
    WA = np.zeros((128, 512), np.float32)
    WB = np.zeros((128, 512), np.float32)
    mb1p = np.zeros((128, H), np.float32)
    for h in range(H):
        sl = slice(128 * h, 128 * (h + 1))
        c1_h = 1.0 + ivb2f[:HD][sl]
        bv_h = f["bv"][sl]
        b2_h = ivb2f[HD:][sl]
        WA[:, sl] = f["wv"][:, sl] @ np.diag(c1_h) @ f["mw1"]
        WB[:, sl] = (ivw2g[:, sl] @ np.diag(bv_h) + ivw2b[:, sl]) @ f["mw1"]
        mb1p[:, h] = f["mb1"] + (bv_h * c1_h + b2_h) @ f["mw1"]

    mw2f = f["mls"][:, None] * f["mw2"]
    mb2f = f["mb2"] + f["mlb"] @ f["mw2"]
    Wmo = np.zeros((128, 512), np.float32)
    wmo1m = np.zeros((128, 512), np.float32)
    for h in range(H):
        wo_h = f["wo"][128 * h : 128 * (h + 1), :]
        Wmo_h = mw2f @ wo_h
        Wmo[:, 128 * h : 128 * (h + 1)] = Wmo_h
        wmo1m[32 * h, 128 * h : 128 * (h + 1)] = -Wmo_h.sum(axis=0)
    bmo = f["bo"] + sum(mb2f @ f["wo"][128 * h : 128 * (h + 1), :] for h in range(H))

    cw2f = f["cls"][:, None] * f["cw2"]
    cb2f = f["cb2"] + f["clb"] @ f["cw2"]

    weights = {
        "rff": np.ascontiguousarray(rff),
        "ws1": bf(ws1),
        "wc1": bf(wc1),
        "eq_b1": col(f["eq_b1"]),
        "Mq": bf(Mq),
        "w1v": bf(w1v),
        "w2v": bf(w2v),
        "attconst": attconst.astype(np.float32),
        "ev_b1": col(f["ev_b1"]),
        "ev_w2": bf(f["ev_w2"]),
        "ev_b2": col(f["ev_b2"]),
        "ivw1": bf(f["ivw1"]),
        "ivb1": col(f["ivb1"]),
        "ivw2g": bf(ivw2g),
        "wv": bf(f["wv"]),
        "WA": bf(WA),
        "WB": bf(WB),
        "mw1": bf(f["mw1"]),
        # mFFN gelu is computed as silu(1.702*(m1+mb1p)) = 1.702*gelu_sig(m1+mb1p)
        # (the 1.702 factor cancels through the LN rstd), so pre-scale the bias
        "mb1p": np.ascontiguousarray(1.702 * mb1p),
        "Wmo": bf(Wmo),
        "wmo1m": bf(wmo1m),
        "bmo": col(bmo),
        "cw1": bf(f["cw1"]),
        "cb1": col(f["cb1"]),
        "cw2g": bf(cw2f[:, :DH]),
        "cw2b": bf(cw2f[:, DH:]),
        "cb2g1": col(cb2f[:DH] + 1.0),
        "cb2b": col(cb2f[DH:]),
    }

    x_flat = f["x"].reshape(B * N, CD)
    xh_flat = f["x_h"].reshape(B * N, DH)

    in_maps = []
    for i in range(NCORES):
        b = (i * NQ) // N
        rs = slice(i * NQ, (i + 1) * NQ)
        p_b = f["p"][b]
        c_b = f["c"][b]
        sig_b = f["window_sigma"][b]
        inv2 = (1.0 / (sig_b[:, 0] ** 2)).astype(np.float16)
        pb = (p_b @ rff).astype(np.float32)
        pbf = (pb - np.rint(pb)).astype(np.float16)
        gtbl = np.zeros((L, 3 * D), ml_dtypes.bfloat16)
        gtbl[:, :D] = bf(c_b)
        gtbl[:, D : 2 * D] = pbf.view(ml_dtypes.bfloat16)
        gtbl[:, 2 * D] = inv2.view(ml_dtypes.bfloat16)
        m = {
            "x": np.ascontiguousarray(x_flat[rs]),
            "xh": np.ascontiguousarray(xh_flat[rs]),
            "gtbl": gtbl,
            "p2t": np.ascontiguousarray((2.0 * p_b).T),
            "npsq": np.ascontiguousarray(-(p_b**2).sum(1)[None, :]),
        }
        m.update(weights)
        in_maps.append(m)
    return in_maps


_PROGRAM_CACHE = {}


def kernel(**inputs):
    in_maps = _host_prep(inputs)
    if "nc" not in _PROGRAM_CACHE:
        _PROGRAM_CACHE["nc"] = build_program()
    nc = _PROGRAM_CACHE["nc"]

    from concourse.bass_utils import run_bass_kernel_spmd

    res = run_bass_kernel_spmd(nc, in_maps, core_ids=list(range(NCORES)))
    outs = [np.asarray(res.results[i]["out"], np.float32) for i in range(NCORES)]
    return np.concatenate(outs, axis=0).reshape(B, N, DH)



# revision 31
# speedup vs baseline: 1.0601x; 1.0540x over previous
"""EquivariantCrossAttention Trainium2 kernel (8 NeuronCores, SPMD) — v2.

kernel(**inputs) takes the FULL unsharded inputs from reference's
setup_inputs() and returns the FULL (B, N, DH) float32 output.

Sharding: flattened query axis (B*N = 4096) split into 8 shards of 512
queries; core i gets queries [512*i, 512*(i+1)) plus its batch's latent
tables. Weights replicated.

Hardcoded problem shapes: B=2 N=2048 L=1024 K=16 CD=2 H=4 DH=128 HD=512.

v2 was 749.6us. v3 changes:
  - cross-tile ACT-table phase pipeline (4 set loads/tile instead of ~9.5):
    [silu: sins(t)+gms(t-1)] [exp: att_e(t-1)] [gelu_tanh: g2/ev/iv1(t)]
    [absrsqrt: iv-rstd(t)+m-rstd(t-1)]; mFFN gelu switched to the sigmoid
    approx via Silu (sin+silu share one table set; 1.702 cancels in LN)
  - small-matmul bursts (LN stats, att reduces, broadcasts) emitted
    back-to-back on distinct row/col groups for PE tile concurrency
  - frac via add_range_wrap custom DVE op (kills int-cast + abs chain);
    cos arg = wrap(tfull + 0.25), so both sins share scale with no bias
  - gsq/vg evacuations moved to ScalarE (Square/Copy are in every set)
"""

import sys

sys.path.insert(0, "/opt/trn_rl_repo")

import numpy as np
import ml_dtypes

import concourse.bass as bass
import concourse.bacc as bacc
import concourse.mybir as mybir
import concourse.tile as tile
from concourse.masks import make_identity

F32 = mybir.dt.float32
BF16 = mybir.dt.bfloat16
F16 = mybir.dt.float16
U32 = mybir.dt.uint32
I32 = mybir.dt.int32
I16 = mybir.dt.int16
AF = mybir.ActivationFunctionType
OP = mybir.AluOpType
AX = mybir.AxisListType

B, N, L, K, CD, H, DH, D = 2, 2048, 1024, 16, 2, 4, 128, 128
HD = H * DH
FQ = 2.0
FV = 2.0
SCALE = 1.0 / float(np.sqrt(DH))
NCORES = 8
NQ = (B * N) // NCORES  # queries per core = 512
QT = NQ // 128  # query tiles per core = 4
NCH = 4  # chunks per query tile
CQ = 128 // NCH  # queries per chunk = 32
CR = CQ * K  # rows per chunk = 512
TR = 128 * K  # rows per tile = 2048
GELU = AF.Gelu_apprx_tanh
TWO_PI = 2.0 * np.pi

WSPECS = [
    ("rff", [CD, 128], F32),
    ("ws1", [128, 128], BF16),   # rows 0:64 eq_w1 sin-half, 64:128 ev_w1 sin-half
    ("wc1", [128, 128], BF16),   # rows 0:64 eq_w1 cos-half, 64:128 ev_w1 cos-half
    ("eq_b1", [128, 1], F32),
    ("Mq", [128, 512], BF16),
    ("w1v", [1, 512], BF16),
    ("w2v", [128, 4], BF16),
    ("attconst", [128, 1], F32),
    ("ev_b1", [128, 1], F32),
    ("ev_w2", [128, 128], BF16),
    ("ev_b2", [128, 1], F32),
    ("ivw1", [128, 128], BF16),
    ("ivb1", [128, 1], F32),
    ("ivw2g", [128, 512], BF16),
    ("wv", [128, 512], BF16),
    ("WA", [128, 512], BF16),
    ("WB", [128, 512], BF16),
    ("mw1", [128, 128], BF16),
    ("mb1p", [128, 4], F32),
    ("Wmo", [128, 512], BF16),
    ("wmo1m", [128, 512], BF16),  # row 32h, cols 128h:128h+128 = -(Wmo_h^T @ 1)
    ("bmo", [128, 1], F32),
    ("cw1", [128, 128], BF16),
    ("cb1", [128, 1], F32),
    ("cw2g", [128, 128], BF16),
    ("cw2b", [128, 128], BF16),
    ("cb2g1", [128, 1], F32),
    ("cb2b", [128, 1], F32),
]


def _bcast_inner(ap, n):
    """[.., Q] AP -> [.., Q, n] with a stride-0 inner dim (free broadcast)."""
    newap = [list(p) for p in ap.ap] + [[0, n]]
    return bass.AP(ap.tensor, ap.offset, newap)


def build_program():
    nc = bacc.Bacc()

    x_d = nc.declare_dram_parameter("x", [NQ, CD], F32, isOutput=False)
    xh_d = nc.declare_dram_parameter("xh", [NQ, DH], F32, isOutput=False)
    gtbl_d = nc.declare_dram_parameter("gtbl", [L, 3 * D], BF16, isOutput=False)
    p2t_d = nc.declare_dram_parameter("p2t", [CD, L], F32, isOutput=False)
    npsq_d = nc.declare_dram_parameter("npsq", [1, L], F32, isOutput=False)
    w_d = {}
    for name, shape, dt in WSPECS:
        w_d[name] = nc.declare_dram_parameter(name, shape, dt, isOutput=False)
    out_d = nc.declare_dram_parameter("out", [NQ, DH], F32, isOutput=True)

    with tile.TileContext(nc) as tc:
        _emit(nc, tc, x_d, xh_d, gtbl_d, p2t_d, npsq_d, w_d, out_d)
    nc.compile()
    return nc


def _emit(nc, tc, x_d, xh_d, gtbl_d, p2t_d, npsq_d, w_d, out_d):
    const = tc.alloc_tile_pool(name="const", bufs=1)
    wpool = tc.alloc_tile_pool(name="wpool", bufs=1)
    core = tc.alloc_tile_pool(name="core", bufs=1)
    hd = tc.alloc_tile_pool(name="hd", bufs=1)   # per-tile head tiles
    tl = tc.alloc_tile_pool(name="tl", bufs=1)   # per-tile body tiles
    ck = tc.alloc_tile_pool(name="ck", bufs=1)   # per-chunk transient tiles
    psp = tc.alloc_tile_pool(name="psp", bufs=1, space="PSUM")
    _pools = [const, wpool, core, hd, tl, ck, psp]

    _psn = [0]

    def PS(shape, tag, bufs, dtype=F32):
        _psn[0] += 1
        return psp.tile(
            shape, dtype, space="PSUM", tag=tag, bufs=bufs, name=f"ps_{tag}_{_psn[0]}"
        )

    # ---------- constants ----------
    ident = const.tile([128, 128], F32)
    make_identity(nc, ident[:])

    ones_col_bf = const.tile([128, 1], BF16)
    nc.vector.memset(ones_col_bf[:], 1.0)
    inv128_bf = const.tile([128, 1], BF16)
    nc.vector.memset(inv128_bf[:], 1.0 / 128.0)
    ones_row_bf = const.tile([1, 128], BF16)
    nc.vector.memset(ones_row_bf[:], 1.0)
    ones_row512_bf = const.tile([1, 512], BF16)
    nc.vector.memset(ones_row512_bf[:], 1.0)
    half_row128_bf = const.tile([1, 128], BF16)
    nc.vector.memset(half_row128_bf[:], 0.5)
    onesmat_bf = const.tile([128, 128], BF16)
    nc.vector.memset(onesmat_bf[:], 1.0)
    zeros_row_bf = const.tile([1, 128], BF16)
    nc.vector.memset(zeros_row_bf[:], 0.0)
    zeros_row512_bf = const.tile([1, 512], BF16)
    nc.vector.memset(zeros_row512_bf[:], 0.0)
    ones_row_f32 = const.tile([1, 128], F32)
    nc.vector.memset(ones_row_f32[:], 1.0)
    pihalf = const.tile([128, 1], F32)
    nc.vector.memset(pihalf[:], np.pi / 2.0)
    eps_col = const.tile([128, 1], F32)
    nc.vector.memset(eps_col[:], 1e-6)

    # ---------- weights ----------
    W = {}
    for name, shape, dt in WSPECS:
        wt = wpool.tile(shape, dt, name=f"w_{name}", tag=f"w_{name}")
        nc.sync.dma_start(out=wt[:], in_=w_d[name][:])
        W[name] = wt

    def Wh(name, h, w=128):
        return W[name][:, h * w : (h + 1) * w]

    # ---------- per-core precompute ----------
    x_sb = core.tile([128, QT, CD], F32)
    nc.sync.dma_start(out=x_sb[:], in_=x_d[:].rearrange("(t q) c -> q t c", q=128))
    xsq = core.tile([128, QT], F32)
    xs2 = core.tile([128, QT, CD], F32)
    nc.vector.tensor_tensor(out=xs2[:], in0=x_sb[:], in1=x_sb[:], op=OP.mult)
    nc.vector.tensor_reduce(out=xsq[:], in_=xs2[:], axis=AX.X, op=OP.add)

    x_fm = core.tile([CD, NQ], F32)
    for t in range(QT):
        tp = PS([CD, 128], "tr", 1)
        nc.tensor.transpose(out=tp[:], in_=x_sb[:, t, :], identity=ident[:])
        nc.vector.tensor_copy(out=x_fm[:, 128 * t : 128 * (t + 1)], in_=tp[:])

    p2_fm = core.tile([CD, L], F32)
    nc.sync.dma_start(out=p2_fm[:], in_=p2t_d[:])
    npsq = core.tile([1, L], F32)
    nc.sync.dma_start(out=npsq[:], in_=npsq_d[:])

    # ---------- cFFN on x_h (512 queries at once) ----------
    xh_fm = core.tile([128, NQ], BF16)
    xh_rm = core.tile([128, QT, DH], F32)
    nc.sync.dma_start(out=xh_rm[:], in_=xh_d[:].rearrange("(t q) c -> q t c", q=128))
    for t in range(QT):
        tp = PS([128, 128], "tr", 1)
        nc.tensor.transpose(out=tp[:], in_=xh_rm[:, t, :], identity=ident[:])
        nc.vector.tensor_copy(out=xh_fm[:, 128 * t : 128 * (t + 1)], in_=tp[:])

    c1ps = PS([128, NQ], "mm", 5)
    nc.tensor.matmul(out=c1ps[:], lhsT=W["cw1"][:], rhs=xh_fm[:], start=True, stop=True)
    c1 = ck.tile([128, NQ], BF16, tag="ev1", bufs=2)
    nc.scalar.activation(out=c1[:], in_=c1ps[:], func=GELU, bias=W["cb1"][:])

    cmean = PS([128, NQ], "stat", 2)
    nc.tensor.matmul(out=cmean[0:1, :], lhsT=inv128_bf[:], rhs=c1[:], start=True, stop=True)
    c1sq = ck.tile([128, NQ], BF16, tag="vg_sb", bufs=1)
    nc.vector.tensor_tensor(out=c1sq[:], in0=c1[:], in1=c1[:], op=OP.mult)
    cmsq = PS([128, NQ], "stat", 2)
    nc.tensor.matmul(out=cmsq[0:1, :], lhsT=inv128_bf[:], rhs=c1sq[:], start=True, stop=True)

    cm2 = core.tile([1, NQ], F32)
    nc.scalar.square(out=cm2[:], in_=cmean[0:1, :])
    cvar = core.tile([1, NQ], F32)
    nc.vector.tensor_tensor(out=cvar[:], in0=cmsq[0:1, :], in1=cm2[:], op=OP.subtract)
    crstd_bf = core.tile([1, NQ], BF16)
    nc.scalar.activation(
        out=crstd_bf[:], in_=cvar[:], func=AF.Abs_reciprocal_sqrt, bias=eps_col[0:1, :]
    )
    cmr_bf = core.tile([1, NQ], BF16)
    nc.vector.tensor_tensor(out=cmr_bf[:], in0=cmean[0:1, :], in1=crstd_bf[:], op=OP.mult)
    crbc = PS([128, NQ], "mm", 5)
    nc.tensor.matmul(out=crbc[:], lhsT=ones_row_bf[:], rhs=crstd_bf[:], start=True, stop=True)
    cmbc = PS([128, NQ], "mm", 5)
    nc.tensor.matmul(out=cmbc[:], lhsT=ones_row_bf[:], rhs=cmr_bf[:], start=True, stop=True)
    z1c = ck.tile([128, NQ], BF16, tag="z1", bufs=1)
    nc.vector.tensor_tensor(out=z1c[:], in0=c1[:], in1=crbc[:], op=OP.mult)
    zc = ck.tile([128, NQ], BF16, tag="p_sb", bufs=2)
    nc.vector.tensor_tensor(out=zc[:], in0=z1c[:], in1=cmbc[:], op=OP.subtract)

    gp1 = core.tile([128, NQ], BF16)
    modadd = core.tile([128, NQ], BF16)
    gps_ = PS([128, NQ], "mm", 5)
    nc.tensor.matmul(out=gps_[:], lhsT=W["cw2g"][:], rhs=zc[:], start=True, stop=True)
    nc.scalar.activation(out=gp1[:], in_=gps_[:], func=AF.Identity, bias=W["cb2g1"][:])
    btps = PS([128, NQ], "mm", 5)
    nc.tensor.matmul(out=btps[:], lhsT=W["cw2b"][:], rhs=zc[:], start=True, stop=True)
    bt = ck.tile([128, NQ], BF16, tag="ivsq", bufs=2)
    nc.scalar.activation(out=bt[:], in_=btps[:], func=AF.Identity, bias=W["cb2b"][:])
    ma1 = ck.tile([128, NQ], BF16, tag="zp", bufs=2)
    nc.vector.tensor_scalar(
        out=ma1[:], in0=gp1[:], scalar1=W["ev_b2"][:], scalar2=None, op0=OP.mult
    )
    nc.vector.tensor_tensor(out=modadd[:], in0=ma1[:], in1=bt[:], op=OP.add)

    # ================= per query tile: head phase =================
    # scores -> top-16 -> idx16 -> combined gather -> t_x frac
    def emit_head(t):
        qs = slice(128 * t, 128 * (t + 1))

        scores = hd.tile([128, L], F32, tag="scores", bufs=1, name=f"scores_{t}")
        for s in range(2):
            sl = slice(512 * s, 512 * (s + 1))
            scps = PS([128, 512], "mm", 5)
            nc.tensor.matmul(out=scps[:], lhsT=x_fm[:, qs], rhs=p2_fm[:, sl], start=True, stop=False)
            nc.tensor.matmul(out=scps[:], lhsT=ones_row_f32[:], rhs=npsq[:, sl], start=False, stop=True)
            nc.scalar.copy(out=scores[:, sl], in_=scps[:])
        vals = hd.tile([128, K], F32, tag="vals", bufs=2, name=f"vals_{t}")
        idxs = hd.tile([128, K], U32, tag="idxs", bufs=2, name=f"idxs_{t}")
        scr2 = hd.tile([128, L], F32, tag="scr2", bufs=1, name=f"scr2_{t}")
        nc.vector.max(out=vals[:, 0:8], in_=scores[:])
        nc.vector.max_index(out=idxs[:, 0:8], in_max=vals[:, 0:8], in_values=scores[:])
        nc.vector.match_replace(
            out=scr2[:], in_to_replace=vals[:, 0:8], in_values=scores[:], imm_value=-1e30
        )
        nc.vector.max(out=vals[:, 8:16], in_=scr2[:])
        nc.vector.max_index(out=idxs[:, 8:16], in_max=vals[:, 8:16], in_values=scr2[:])

        # index prep for dma_gather: idx16[k, q] (int16) replicated across
        # the 8 gpsimd cores' 16-partition blocks
        idxf = hd.tile([128, K], F32, tag="idxf", bufs=1, name=f"idxf_{t}")
        nc.vector.tensor_copy(out=idxf[:], in_=idxs[:])
        idxt_ps = PS([K, 128], "tr", 1)
        nc.tensor.transpose(out=idxt_ps[:], in_=idxf[:], identity=ident[:])
        idx16 = hd.tile([128, 128], I16, tag="idx16", bufs=2, name=f"idx16_{t}")
        nc.vector.tensor_copy(out=idx16[0:16, :], in_=idxt_ps[:])
        for b in range(1, 8):
            nc.sync.dma_start(out=idx16[16 * b : 16 * b + 16, :], in_=idx16[0:16, :])

        # ONE combined transposed gather for the whole tile (2048 rows):
        # G[128, 0, :] = c (bf16), G[128, 1, :] = frac(p@Bs) (f16 bits),
        # G[0, 2, :] = 1/sigma^2 (f16 bits)
        Gs = []
        for c in range(NCH):
            G = hd.tile([128, 3, CR], BF16, tag=f"G{c}", bufs=2, name=f"G{c}_{t}")
            nc.gpsimd.dma_gather(
                out_ap=G[:], in_ap=gtbl_d[:],
                idxs_ap=idx16[:, 32 * c : 32 * c + 32], num_idxs=CR,
                num_idxs_reg=CR, elem_size=3 * D, transpose=True,
            )
            Gs.append(G)

        # -d^2 row [1, 2048] (q-major) via sbuf->sbuf DMA flatten
        negd2 = hd.tile([128, K], BF16, tag="negd2", bufs=2, name=f"negd2_{t}")
        nc.vector.tensor_scalar(
            out=negd2[:], in0=vals[:], scalar1=xsq[:, t : t + 1], scalar2=None,
            op0=OP.subtract,
        )
        negd2_row = hd.tile([1, TR], BF16, tag="negd2_row", bufs=2, name=f"nd2r_{t}")
        nc.sync.dma_start(out=negd2_row[:], in_=negd2[:])

        # t_x = x @ Bs for this tile, pre-frac'd: txf = t_x - rint(t_x), f16
        txps = PS([128, 128], "tr", 1)
        nc.tensor.matmul(out=txps[:], lhsT=W["rff"][:], rhs=x_fm[:, qs], start=True, stop=True)
        t_x = hd.tile([128, 128], F32, tag="t_x", bufs=1, name=f"t_x_{t}")
        nc.vector.tensor_copy(out=t_x[:], in_=txps[:])
        txi = hd.tile([128, 128], I32, tag="txi", bufs=1, name=f"txi_{t}")
        nc.vector.tensor_copy(out=txi[:], in_=t_x[:])
        txf = hd.tile([128, 128], F32, tag="txf", bufs=2, name=f"txf_{t}")
        nc.vector.tensor_tensor(out=txf[:], in0=t_x[:], in1=txi[:], op=OP.subtract)
        return dict(Gs=Gs, negd2_row=negd2_row, txf=txf)

    # ================= per query tile: phased body =================
    # ACT table-set phases per steady-state iteration t:
    #   [silu]      sins(t) + mFFN gelus gms(t-1)     (Sin+Silu share a set)
    #   [gelu_tanh] g2/ev1/iv1 gelus (t) + att logits
    #   [absrsqrt]  iv-rstd(t) + m-rstd(t-1)
    #   [exp]       softmax att_e(t-1), then P10/P11(t-1) (no ACT sets)
    # Copy/Identity/Square/Abs are in every set (free).

    state = {}

    def emit_sins(t, ht):
        Gs = ht["Gs"]
        negd2_row = ht["negd2_row"]
        txf = ht["txf"]

        # ---- RFF frac chain (DVE) ----
        # tfull = txf - pbf in [-1, 1]; range-wrap replaces the int-cast
        # round: fs = wrap(tfull) in [-.5, .5]; fc = wrap(tfull + .25) so
        # sin(2*pi*fc) = cos(2*pi*tfull) with no abs / bias needed.
        fs = tl.tile([128, TR], F16, tag="fs", bufs=1, name=f"fs_{t}")
        fc = tl.tile([128, TR], F16, tag="fc", bufs=1, name=f"fc_{t}")
        for c in range(NCH):
            cs = slice(CR * c, CR * (c + 1))
            tfc = ck.tile([128, CR], F16, tag="tfc", bufs=2)
            nc.vector.tensor_tensor(
                out=tfc[:].rearrange("p (a b) -> p a b", a=CQ),
                in0=_bcast_inner(txf[:, CQ * c : CQ * (c + 1)], K),
                in1=Gs[c][:, 1, :].bitcast(F16).rearrange("p (a b) -> p a b", a=CQ),
                op=OP.subtract,
            )
            nc.vector.add_range_wrap(out=fs[:, cs], in_=tfc[:], shift=0.0, bound=0.5, period=1.0)
            nc.vector.add_range_wrap(out=fc[:, cs], in_=tfc[:], shift=0.25, bound=0.5, period=1.0)
        # gw row [1, 2048] = -d^2 * (1/sigma^2)
        gw_row = tl.tile([1, TR], BF16, tag="gw_row", bufs=1, name=f"gw_{t}")
        for c in range(NCH):
            cs = slice(CR * c, CR * (c + 1))
            nc.vector.tensor_tensor(
                out=gw_row[:, cs], in0=negd2_row[:, cs],
                in1=Gs[c][0:1, 2, :].bitcast(F16), op=OP.mult,
            )

        # ---- sins (silu set) ----
        sfull = tl.tile([128, TR], BF16, tag="sfull", bufs=1, name=f"sfull_{t}")
        cfull = tl.tile([128, TR], BF16, tag="cfull", bufs=1, name=f"cfull_{t}")
        nc.scalar.activation(out=sfull[:], in_=fs[:], func=AF.Sin, scale=TWO_PI)
        nc.scalar.activation(out=cfull[:], in_=fc[:], func=AF.Sin, scale=TWO_PI)
        st = state.setdefault(t, {})
        st.update(sfull=sfull, cfull=cfull, gw_row=gw_row, Gs=Gs)

    def emit_geluA(t):
        st = state[t]
        sfull, cfull, gw_row, Gs = st["sfull"], st["cfull"], st["gw_row"], st["Gs"]

        def cg(c):
            return Gs[c][:, 0, :]

        # ---- q path + attention logits ----
        # all in-loop gelus use the sigmoid approx via Silu so they share
        # one ACT table set with Sin; consumer weights absorb 1/1.702
        g2 = tl.tile([128, TR], BF16, tag="g2", bufs=1, name=f"g2_{t}")
        att_l = tl.tile([128, TR], BF16, tag="att_l", bufs=1, name=f"attl_{t}")
        iv1 = tl.tile([128, TR], BF16, tag="iv1", bufs=1, name=f"iv1_{t}")
        # ivs33 rows: 0 = irstd (later), 32 = imr (later)
        ivs33 = tl.tile([33, TR], BF16, tag="ivs33", bufs=1, name=f"ivs33_{t}")
        ivmean_row = tl.tile([1, TR], BF16, tag="ivmean_row", bufs=1, name=f"ivmr_{t}")
        ivar_all = tl.tile([1, TR], BF16, tag="ivar_all", bufs=1, name=f"ivva_{t}")
        for c in range(NCH):
            cs = slice(CR * c, CR * (c + 1))
            gq = slice(128 * t + CQ * c, 128 * t + CQ * (c + 1))
            # g2 (rows 0:64) and ev1 (rows 64:128) interleaved: disjoint
            # row-groups let the PE overlap the pairs
            g2ps = PS([128, CR], "mm", 5)
            ev1ps = PS([128, CR], "mm", 5)
            nc.tensor.matmul(out=g2ps[:], lhsT=W["ws1"][0:64, :], rhs=sfull[0:64, cs], start=True, stop=False)
            nc.tensor.matmul(out=ev1ps[:], lhsT=W["ws1"][64:128, :], rhs=sfull[64:128, cs], start=True, stop=False)
            nc.tensor.matmul(out=g2ps[:], lhsT=W["wc1"][0:64, :], rhs=cfull[0:64, cs], start=False, stop=True)
            nc.tensor.matmul(out=ev1ps[:], lhsT=W["wc1"][64:128, :], rhs=cfull[64:128, cs], start=False, stop=True)
            nc.scalar.activation(out=g2[:, cs], in_=g2ps[:], func=AF.Silu, scale=1.702, bias=W["eq_b1"][:])
            ev1 = ck.tile([128, CR], BF16, tag="ev1", bufs=2)
            nc.scalar.activation(out=ev1[:], in_=ev1ps[:], func=AF.Silu, scale=1.702, bias=W["ev_b1"][:])

            # attention: 4 full MMs evacuated by ScalarE, qkp in-place on
            # the evac tile (both-SBUF bf16 -> 2x DVE mode), then the 8
            # small reduce MMs back-to-back on distinct col-groups
            attps = PS([128, CR], "stat", 2)
            nc.tensor.matmul(
                out=attps[:], lhsT=half_row128_bf[:],
                rhs=gw_row[:, cs], start=True, stop=False,
                skip_group_check=True,
            )
            qkp = ck.tile([128, H, CR], BF16, tag="qkp", bufs=1)
            for h in range(H):
                ups = PS([128, CR], "mm", 5)
                nc.tensor.matmul(out=ups[:], lhsT=Wh("Mq", h), rhs=g2[:, cs], start=True, stop=False)
                nc.tensor.matmul(
                    out=ups[:], lhsT=W["w1v"][:, 128 * h : 128 * (h + 1)],
                    rhs=ones_row512_bf[:], start=False, stop=True,
                )
                nc.scalar.copy(out=qkp[:, h, :], in_=ups[:])
            for h in range(H):
                nc.vector.tensor_tensor(out=qkp[:, h, :], in0=qkp[:, h, :], in1=cg(c), op=OP.mult)
            for h in range(H):
                nc.tensor.matmul(
                    out=attps[32 * h : 32 * h + 1, :], lhsT=ones_col_bf[:], rhs=qkp[:, h, :],
                    start=False, stop=False, tile_position=(0, 32 * h),
                    skip_group_check=True,
                )
            for h in range(H):
                nc.tensor.matmul(
                    out=attps[32 * h : 32 * h + 1, :], lhsT=W["w2v"][:, h : h + 1],
                    rhs=g2[:, cs], start=False, stop=(h == H - 1), tile_position=(0, 32 * h),
                    skip_group_check=True,
                )
            nc.scalar.copy(out=att_l[:, cs], in_=attps[:])

            # ---- ev modulation -> iv1 ----
            ev2ps = PS([128, CR], "mm", 5)
            nc.tensor.matmul(out=ev2ps[:], lhsT=W["ev_w2"][:], rhs=ev1[:], start=True, stop=True)
            mv = ck.tile([128, CQ, K], BF16, tag="mv", bufs=1)
            nc.vector.tensor_tensor(
                out=mv[:], in0=ev2ps[:].rearrange("p (a b) -> p a b", a=CQ),
                in1=_bcast_inner(gp1[:, gq], K), op=OP.mult,
            )
            ivin = ck.tile([128, CQ, K], BF16, tag="ivin", bufs=2)
            nc.vector.tensor_tensor(
                out=ivin[:], in0=mv[:], in1=_bcast_inner(modadd[:, gq], K), op=OP.add
            )
            iv1ps = PS([128, CR], "mm", 5)
            nc.tensor.matmul(
                out=iv1ps[:], lhsT=W["ivw1"][:],
                rhs=ivin[:].rearrange("p a b -> p (a b)"), start=True, stop=True,
            )
            nc.scalar.activation(out=iv1[:, cs], in_=iv1ps[:], func=AF.Silu, scale=1.702, bias=W["ivb1"][:])

            # ---- iv LN stats (PE + in-set Square/Copy); rstd deferred ----
            ivst = PS([128, CR], "stat", 2)
            ivsq = ck.tile([128, CR], BF16, tag="ivsq", bufs=2)
            nc.vector.tensor_tensor(out=ivsq[:], in0=iv1[:, cs], in1=iv1[:, cs], op=OP.mult)
            nc.tensor.matmul(
                out=ivst[0:1, :], lhsT=inv128_bf[:], rhs=iv1[:, cs],
                start=True, stop=True, skip_group_check=True,
            )
            nc.tensor.matmul(
                out=ivst[32:33, :], lhsT=inv128_bf[:], rhs=ivsq[:],
                start=True, stop=True, tile_position=(0, 32),
                skip_group_check=True,
            )
            # mean parked at ivs33 row 16 (DVE handles the partition shift)
            nc.vector.tensor_copy(out=ivmean_row[:, cs], in_=ivst[0:1, :])
            im2 = ck.tile([1, CR], BF16, tag="im2")
            nc.scalar.square(out=im2[:], in_=ivst[0:1, :])
            nc.vector.tensor_tensor(
                out=ivar_all[:, cs], in0=ivst[32:33, :], in1=im2[:], op=OP.subtract
            )
        st.update(g2=g2, att_l=att_l, iv1=iv1, ivs33=ivs33, ivmean_row=ivmean_row, ivar_all=ivar_all)

    def emit_ivrstd(t):
        # [absrsqrt set] one rstd for the whole tile, then LN apply (DVE/PE)
        st = state[t]
        iv1, ivs33, ivar_all = st["iv1"], st["ivs33"], st["ivar_all"]
        ivmean_row = st["ivmean_row"]
        nc.scalar.activation(
            out=ivs33[0:1, :], in_=ivar_all[:], func=AF.Abs_reciprocal_sqrt,
            bias=eps_col[0:1, :],
        )
        # imr at partition 32 so its broadcast MM uses row-group 1 (overlaps
        # with irbc's row-group 0)
        nc.vector.tensor_tensor(
            out=ivs33[32:33, :], in0=ivmean_row[:], in1=ivs33[0:1, :], op=OP.mult
        )
        ziv = tl.tile([128, TR], BF16, tag="ziv", bufs=1, name=f"ziv_{t}")
        for c in range(NCH):
            cs = slice(CR * c, CR * (c + 1))
            irbc = PS([128, CR], "mm", 5)
            imbc = PS([128, CR], "mm", 5)
            nc.tensor.matmul(out=irbc[:], lhsT=ones_row_bf[:], rhs=ivs33[0:1, cs], start=True, stop=True)
            nc.tensor.matmul(
                out=imbc[:], lhsT=onesmat_bf[32:33, :], rhs=ivs33[32:33, cs],
                start=True, stop=True, tile_position=(32, 0),
            )
            z1 = ck.tile([128, CR], BF16, tag="z1", bufs=1)
            nc.vector.tensor_tensor(out=z1[:], in0=iv1[:, cs], in1=irbc[:], op=OP.mult)
            nc.vector.tensor_tensor(out=ziv[:, cs], in0=z1[:], in1=imbc[:], op=OP.subtract)
        st["ziv"] = ziv

    def emit_gms(t):
        # [silu set] mFFN: gelu via silu(1.702x) (=1.702*gelu_sig(x); the
        # 1.702 factor cancels exactly through the LN rstd)
        st = state[t]
        ziv, Gs = st["ziv"], st["Gs"]

        def cg(c):
            return Gs[c][:, 0, :]

        gms = [
            tl.tile([128, TR], BF16, tag=f"gm{h}", bufs=1, name=f"gm{h}_{t}")
            for h in range(H)
        ]
        mvar_all = tl.tile([128, TR], BF16, tag="mvar", bufs=1, name=f"mvar_{t}")
        mmean_sb = tl.tile([128, TR], BF16, tag="mmean_sb", bufs=1, name=f"mmean_{t}")
        for c in range(NCH):
            cs = slice(CR * c, CR * (c + 1))
            mmean = PS([128, CR], "stat", 2)
            msqp = PS([128, CR], "stat", 2)
            gsq = ck.tile([128, H, CR], BF16, tag="gsq", bufs=1)
            for h in range(H):
                vgps = PS([128, CR], "mm", 5)
                nc.tensor.matmul(out=vgps[:], lhsT=Wh("ivw2g", h), rhs=ziv[:, cs], start=True, stop=True)
                vg_sb = ck.tile([128, CR], BF16, tag="vg_sb", bufs=1)
                nc.scalar.copy(out=vg_sb[:], in_=vgps[:])
                v0ps = PS([128, CR], "mm", 5)
                nc.tensor.matmul(out=v0ps[:], lhsT=Wh("wv", h), rhs=cg(c), start=True, stop=True)
                p_sb = ck.tile([128, CR], BF16, tag="p_sb", bufs=2)
                nc.scalar.copy(out=p_sb[:], in_=v0ps[:])
                nc.vector.tensor_tensor(out=p_sb[:], in0=p_sb[:], in1=vg_sb[:], op=OP.mult)
                m1ps = PS([128, CR], "mm", 5)
                nc.tensor.matmul(out=m1ps[:], lhsT=W["mw1"][:], rhs=p_sb[:], start=True, stop=False)
                nc.tensor.matmul(out=m1ps[:], lhsT=Wh("WA", h), rhs=cg(c), start=False, stop=False)
                nc.tensor.matmul(out=m1ps[:], lhsT=Wh("WB", h), rhs=ziv[:, cs], start=False, stop=True)
                nc.scalar.activation(
                    out=gms[h][:, cs], in_=m1ps[:], func=AF.Silu, scale=1.702,
                    bias=W["mb1p"][:, h : h + 1],
                )
                nc.vector.tensor_tensor(out=gsq[:, h, :], in0=gms[h][:, cs], in1=gms[h][:, cs], op=OP.mult)
            # stat MMs in two 4-bursts (col-groups 0..3 back-to-back)
            for h in range(H):
                nc.tensor.matmul(
                    out=mmean[32 * h : 32 * h + 1, :], lhsT=inv128_bf[:], rhs=gms[h][:, cs],
                    start=True, stop=True, tile_position=(0, 32 * h),
                    skip_group_check=True,
                )
            for h in range(H):
                nc.tensor.matmul(
                    out=msqp[32 * h : 32 * h + 1, :], lhsT=inv128_bf[:], rhs=gsq[:, h, :],
                    start=True, stop=True, tile_position=(0, 32 * h),
                    skip_group_check=True,
                )
            mm2 = ck.tile([128, CR], BF16, tag="mm2")
            nc.scalar.square(out=mm2[:], in_=mmean[:])
            nc.vector.tensor_tensor(out=mvar_all[:, cs], in0=msqp[:], in1=mm2[:], op=OP.subtract)
            nc.scalar.copy(out=mmean_sb[:, cs], in_=mmean[:])
        st.update(gms=gms, mvar_all=mvar_all, mmean_sb=mmean_sb)

    def emit_mrstd(t):
        # [absrsqrt set] — in place: mvar_all becomes mrstd, mmean_sb becomes mmr
        st = state[t]
        mrstd = st["mvar_all"]
        mmr = st["mmean_sb"]
        nc.scalar.activation(
            out=mrstd[:], in_=mrstd[:], func=AF.Abs_reciprocal_sqrt,
            bias=eps_col[:],
        )
        nc.vector.tensor_tensor(out=mmr[:], in0=mmr[:], in1=mrstd[:], op=OP.mult)
        st.update(mrstd=mrstd, mmr=mmr)

    def emit_exp(t):
        # [exp set] softmax numerator
        st = state[t]
        att_e = tl.tile([128, TR], BF16, tag="att_e", bufs=1, name=f"atte_{t}")
        nc.scalar.activation(out=att_e[:], in_=st["att_l"][:], func=AF.Exp, bias=W["attconst"][:])
        st["att_e"] = att_e

    def emit_out(t):
        # P10/P11 (no new ACT sets: Identity only)
        st = state[t]
        att_e, mrstd, mmr, gms = st["att_e"], st["mrstd"], st["mmr"], st["gms"]
        den_t = tl.tile([128, 128], F32, tag="den_t", name=f"den_{t}")
        nc.vector.tensor_reduce(
            out=den_t[:], in_=att_e[:].rearrange("p (a b) -> p a b", a=128),
            axis=AX.X, op=OP.add,
        )
        rden_t = tl.tile([128, 128], F32, tag="rden_t", name=f"rden_{t}")
        nc.vector.reciprocal(out=rden_t[:], in_=den_t[:])
        rdbf = tl.tile([128, 128], BF16, tag="rdbf", name=f"rdbf_{t}")
        nc.vector.tensor_copy(out=rdbf[:], in_=rden_t[:])

        a2 = tl.tile([128, TR], BF16, tag="a2", name=f"a2_{t}")
        nc.vector.tensor_tensor(out=a2[:], in0=att_e[:], in1=mrstd[:], op=OP.mult)
        # a3 in place over att_e (att_e's last readers are den and a2)
        a3 = att_e
        nc.vector.tensor_tensor(out=a3[:], in0=att_e[:], in1=mmr[:], op=OP.mult)
        s3 = tl.tile([128, 128], F32, tag="s3", name=f"s3_{t}")
        nc.vector.tensor_reduce(
            out=s3[:], in_=a3[:].rearrange("p (a b) -> p a b", a=128), axis=AX.X, op=OP.add
        )
        s3rb = tl.tile([128, 128], BF16, tag="s3rb", name=f"s3rb_{t}")
        nc.vector.tensor_tensor(out=s3rb[:], in0=s3[:], in1=rden_t[:], op=OP.mult)

        # fold 1/den into a2 (in place, k-broadcast) so zacc needs no rescale
        nc.vector.tensor_tensor(
            out=a2[:].rearrange("p (a b) -> p a b", a=128),
            in0=a2[:].rearrange("p (a b) -> p a b", a=128),
            in1=_bcast_inner(rdbf[:, :], K), op=OP.mult,
        )

        zaccb = [
            tl.tile([128, 128], BF16, tag=f"zaccb{h}", name=f"zaccb{h}_{t}", bufs=2)
            for h in range(H)
        ]
        zaccs = [
            tl.tile([128, 128], F32, tag=f"zacc{h}", name=f"zacc{h}_{t}")
            for h in range(H)
        ]
        for c in range(NCH):
            cs = slice(CR * c, CR * (c + 1))
            qsl = slice(CQ * c, CQ * (c + 1))
            # 4 broadcast MMs back-to-back (row-groups 0..3: concurrent)
            a2bcs = []
            for h in range(H):
                a2bc = PS([128, CR], "mm", 5)
                nc.tensor.matmul(
                    out=a2bc[:], lhsT=onesmat_bf[32 * h : 32 * h + 1, :],
                    rhs=a2[32 * h : 32 * h + 1, cs], start=True, stop=True,
                    tile_position=(32 * h, 0),
                )
                a2bcs.append(a2bc)
            for h in range(H):
                zp = ck.tile([128, CR], BF16, tag="zp", bufs=2)
                nc.vector.tensor_tensor(out=zp[:], in0=gms[h][:, cs], in1=a2bcs[h][:], op=OP.mult)
                nc.vector.tensor_reduce(
                    out=zaccs[h][:, qsl], in_=zp[:].rearrange("p (a b) -> p a b", a=CQ),
                    axis=AX.X, op=OP.add,
                )
        for h in range(H):
            nc.vector.tensor_copy(out=zaccb[h][:], in_=zaccs[h][:])

        outps = PS([128, 128], "tr", 1)
        for h in range(H):
            nc.tensor.matmul(
                out=outps[:], lhsT=Wh("Wmo", h), rhs=zaccb[h][:], start=(h == 0), stop=False,
                skip_group_check=True,
            )
            nc.tensor.matmul(
                out=outps[:], lhsT=W["wmo1m"][32 * h : 32 * h + 1, 128 * h : 128 * (h + 1)],
                rhs=s3rb[32 * h : 32 * h + 1, :], start=False, stop=(h == H - 1),
                tile_position=(32 * h, 0), skip_group_check=True,
            )
        outsb = tl.tile([128, 128], F32, tag="outsb")
        nc.scalar.activation(out=outsb[:], in_=outps[:], func=AF.Identity, bias=W["bmo"][:])
        trp = PS([128, 128], "tr", 1)
        nc.tensor.transpose(out=trp[:], in_=outsb[:], identity=ident[:])
        outrm = tl.tile([128, 128], F32, tag="outrm")
        nc.vector.tensor_copy(out=outrm[:], in_=trp[:])
        nc.sync.dma_start(out=out_d[slice(128 * t, 128 * (t + 1)), :], in_=outrm[:])
        del state[t]

    # ---- pipelined emission ----
    # iteration t: [silu: sins(t), gms(t-1)] [exp: att_e(t-1)]
    #             [gelu_tanh: geluA(t)] [absrsqrt: ivr(t), mr(t-1)]
    #             then out(t-1) (no ACT sets) + head(t+2)
    heads = {}
    heads[0] = emit_head(0)
    heads[1] = emit_head(1)
    for t in range(QT):
        emit_sins(t, heads[t])
        del heads[t]
        if t - 1 >= 0:
            emit_gms(t - 1)
            emit_exp(t - 1)
        emit_geluA(t)
        emit_ivrstd(t)
        if t - 1 >= 0:
            emit_mrstd(t - 1)
            emit_out(t - 1)
        if t + 2 < QT:
            heads[t + 2] = emit_head(t + 2)
    # tail: tile QT-1
    emit_gms(QT - 1)
    emit_exp(QT - 1)
    emit_mrstd(QT - 1)
    emit_out(QT - 1)

    for p in reversed(_pools):
        p.release()


# ======================= host side =======================


def _host_prep(inputs):
    f = {k: np.asarray(v, np.float32) for k, v in inputs.items()}

    def bf(x):
        return np.ascontiguousarray(np.asarray(x, np.float32)).astype(ml_dtypes.bfloat16)

    def col(x):
        return np.ascontiguousarray(np.asarray(x, np.float32).reshape(-1, 1))

    rff = np.concatenate([FQ * f["rffq"], FV * f["rffv"]], axis=1)  # [2,128]

    wq_s = f["wq"] * SCALE
    bq_s = f["bq"] * SCALE
    W_qm = f["eq_w2"] @ wq_s
    b_qm = f["eq_b2"] @ wq_s + bq_s
    Mq = np.zeros((128, 512), np.float32)
    w1v = np.zeros((1, 512), np.float32)
    w2v = np.zeros((128, 4), np.float32)
    attconst = np.zeros((128, 1), np.float32)
    for h in range(H):
        sl = slice(128 * h, 128 * (h + 1))
        Wq_h = W_qm[:, sl]
        wk_h = f["wk"][:, sl]
        bk_h = f["bk"][sl]
        bq_h = b_qm[sl]
        Mq[:, sl] = Wq_h @ wk_h.T
        w1v[0, sl] = wk_h @ bq_h
        w2v[:, h] = Wq_h @ bk_h
        attconst[32 * h, 0] = float(bq_h @ bk_h)

    # split eq_w1 / ev_w1 into sin/cos input halves
    ws1 = np.zeros((128, 128), np.float32)
    wc1 = np.zeros((128, 128), np.float32)
    ws1[0:64, :] = f["eq_w1"][0:64, :]
    wc1[0:64, :] = f["eq_w1"][64:128, :]
    ws1[64:128, :] = f["ev_w1"][0:64, :]
    wc1[64:128, :] = f["ev_w1"][64:128, :]

    ivw2f = f["ivls"][:, None] * f["ivw2"]
    ivb2f = f["ivb2"] + f["ivlb"] @ f["ivw2"]
    ivw2g = ivw2f[:, :HD]
    ivw2b = ivw2f[:, HD:]
    # bilinear expansion: m1 = mw1.T (v0*vg) + WA.T cg + WB.T ziv + mb1p
    WA = np.zeros((128, 512), np.float32)
    WB = np.zeros((128, 512), np.float32)
    mb1p = np.zeros((128, H), np.float32)
    for h in range(H):
        sl = slice(128 * h, 128 * (h + 1))
        c1_h = 1.0 + ivb2f[:HD][sl]
        bv_h = f["bv"][sl]
        b2_h = ivb2f[HD:][sl]
        WA[:, sl] = f["wv"][:, sl] @ np.diag(c1_h) @ f["mw1"]
        WB[:, sl] = (ivw2g[:, sl] @ np.diag(bv_h) + ivw2b[:, sl]) @ f["mw1"]
        mb1p[:, h] = f["mb1"] + (bv_h * c1_h + b2_h) @ f["mw1"]

    mw2f = f["mls"][:, None] * f["mw2"]
    mb2f = f["mb2"] + f["mlb"] @ f["mw2"]
    Wmo = np.zeros((128, 512), np.float32)
    wmo1m = np.zeros((128, 512), np.float32)
    for h in range(H):
        wo_h = f["wo"][128 * h : 128 * (h + 1), :]
        Wmo_h = mw2f @ wo_h
        Wmo[:, 128 * h : 128 * (h + 1)] = Wmo_h
        wmo1m[32 * h, 128 * h : 128 * (h + 1)] = -Wmo_h.sum(axis=0)
    bmo = f["bo"] + sum(mb2f @ f["wo"][128 * h : 128 * (h + 1), :] for h in range(H))

    cw2f = f["cls"][:, None] * f["cw2"]
    cb2f = f["cb2"] + f["clb"] @ f["cw2"]

    weights = {
        "rff": np.ascontiguousarray(rff),
        "ws1": bf(ws1),
        "wc1": bf(wc1),
        "eq_b1": col(1.702 * f["eq_b1"]),
        "Mq": bf(Mq / 1.702),
        "w1v": bf(w1v),
        "w2v": bf(w2v / 1.702),
        "attconst": attconst.astype(np.float32),
        "ev_b1": col(1.702 * f["ev_b1"]),
        "ev_w2": bf(f["ev_w2"] / 1.702),
        "ev_b2": col(f["ev_b2"]),
        "ivw1": bf(f["ivw1"]),
        "ivb1": col(1.702 * f["ivb1"]),
        "ivw2g": bf(ivw2g),
        "wv": bf(f["wv"]),
        "WA": bf(WA),
        "WB": bf(WB),
        "mw1": bf(f["mw1"]),
        # mFFN gelu is computed as silu(1.702*(m1+mb1p)) = 1.702*gelu_sig(m1+mb1p)
        # (the 1.702 factor cancels through the LN rstd), so pre-scale the bias
        "mb1p": np.ascontiguousarray(1.702 * mb1p),
        "Wmo": bf(Wmo),
        "wmo1m": bf(wmo1m),
        "bmo": col(bmo),
        "cw1": bf(f["cw1"]),
        "cb1": col(f["cb1"]),
        "cw2g": bf(cw2f[:, :DH]),
        "cw2b": bf(cw2f[:, DH:]),
        "cb2g1": col(cb2f[:DH] + 1.0),
        "cb2b": col(cb2f[DH:]),
    }

    x_flat = f["x"].reshape(B * N, CD)
    xh_flat = f["x_h"].reshape(B * N, DH)

    in_maps = []
    for i in range(NCORES):
        b = (i * NQ) // N
        rs = slice(i * NQ, (i + 1) * NQ)
        p_b = f["p"][b]
        c_b = f["c"][b]
        sig_b = f["window_sigma"][b]
        inv2 = (1.0 / (sig_b[:, 0] ** 2)).astype(np.float16)
        pb = (p_b @ rff).astype(np.float32)
        pbf = (pb - np.rint(pb)).astype(np.float16)
        gtbl = np.zeros((L, 3 * D), ml_dtypes.bfloat16)
        gtbl[:, :D] = bf(c_b)
        gtbl[:, D : 2 * D] = pbf.view(ml_dtypes.bfloat16)
        gtbl[:, 2 * D] = inv2.view(ml_dtypes.bfloat16)
        m = {
            "x": np.ascontiguousarray(x_flat[rs]),
            "xh": np.ascontiguousarray(xh_flat[rs]),
            "gtbl": gtbl,
            "p2t": np.ascontiguousarray((2.0 * p_b).T),
            "npsq": np.ascontiguousarray(-(p_b**2).sum(1)[None, :]),
        }
        m.update(weights)
        in_maps.append(m)
    return in_maps


_PROGRAM_CACHE = {}


def kernel(**inputs):
    in_maps = _host_prep(inputs)
    if "nc" not in _PROGRAM_CACHE:
        _PROGRAM_CACHE["nc"] = build_program()
    nc = _PROGRAM_CACHE["nc"]

    from concourse.bass_utils import run_bass_kernel_spmd

    res = run_bass_kernel_spmd(nc, in_maps, core_ids=list(range(NCORES)))
    outs = [np.asarray(res.results[i]["out"], np.float32) for i in range(NCORES)]
    return np.concatenate(outs, axis=0).reshape(B, N, DH)



# revision 37
# speedup vs baseline: 1.1569x; 1.0914x over previous
"""EquivariantCrossAttention Trainium2 kernel (8 NeuronCores, SPMD) — v4.

kernel(**inputs) takes the FULL unsharded inputs from reference's
setup_inputs() and returns the FULL (B, N, DH) float32 output.

Sharding: flattened query axis (B*N = 4096) split into 8 shards of 512
queries; core i gets queries [512*i, 512*(i+1)) plus its batch's latent
tables. Weights replicated.

Hardcoded problem shapes: B=2 N=2048 L=1024 K=16 CD=2 H=4 DH=128 HD=512.

Final: 654.9us HW max-core (v2 baseline 749.6us), rel err 1.55e-2 (gate 2e-2).

v3/v4 changes vs v2:
  - cross-tile ACT-table phase pipeline (23 set loads total vs 35):
    [silu: sins(t)+gms(t-1)+g2/ev/iv1(t)] [exp: att_e(t-1)]
    [absrsqrt: iv-rstd(t)+m-rstd(t-1)]. ALL in-loop gelus use the sigmoid
    approximation via Silu (scale=1.702; consumers absorb 1/1.702, and it
    cancels outright through both LNs) so Sin+Silu share ONE table set.
  - small-matmul bursts (LN stats, att reduces, broadcasts) emitted
    back-to-back on distinct row/col groups for PE tile concurrency
  - frac via add_range_wrap custom DVE op (kills int-cast + abs chain);
    cos arg = wrap(tfull + 0.25) so both sins share scale with no bias
  - PSUM evacuations on ScalarE (Copy in every set); the qkp/p_sb/zp
    products then run in the 2x both-SBUF-bf16 DVE mode
  - head tiles (G gathers, txf, negd2_row) triple-buffered so the
    topk+gather pipeline truly runs 2 tiles ahead of the body
  - softmax 1/den folded into the a2 attention weights (k-broadcast mult)
    instead of per-head rdbc broadcast matmuls

v5-v8 changes:
  - w1v (= wk_h @ bq_h, a per-partition constant) applied as the ScalarE
    evacuation bias of ups instead of a broadcast matmul (-64 MMs/core)
  - zp runs in 2x DVE mode: a2bc evacuated by ScalarE into the zp tile,
    then multiplied in place (reverting this measured 668us - keep it)
  - emit_head(t+2) moved right after sins(t) so topk/gathers start ~a full
    phase earlier and the scores MMs fill mid-iteration PE gaps
  - PSUM pools mm=5/stat=2/tr=1 (mm=4/stat=3 measured 804us - the shared
    mm pool is the binding PSUM resource; don't shrink it)

Known remaining bottleneck (for future work): the Tensor engine is ~90%
occupied but HAM-throttled to 1.2 GHz for ~half the span (micro-gaps from
PSUM-evac round trips keep re-throttling it). ~400 of the 958 matmuls are
reduction/broadcast overhead; merging them hits the one-PSUM-bank output
limit (1-row x N outputs can't exceed 512 f32). Candidates: fp8 DoubleRow
for the mFFN m1 trio (~25us, ~5-8e-3 extra error), gather rows 768->512B
by packing 1/sigma^2 into a pb byte (~50us GpSimd). Failed experiments:
stride-0-out matmul K-reduce (hard device crash), DMA partition-broadcast
(AP rejected), indirect_copy sigma LUT (indices broadcast per 16-part
group). Profile helper: analyze.py.
"""

import sys

sys.path.insert(0, "/opt/trn_rl_repo")

import numpy as np
import ml_dtypes

import concourse.bass as bass
import concourse.bacc as bacc
import concourse.mybir as mybir
import concourse.tile as tile
from concourse.masks import make_identity

F32 = mybir.dt.float32
BF16 = mybir.dt.bfloat16
F16 = mybir.dt.float16
U32 = mybir.dt.uint32
I32 = mybir.dt.int32
I16 = mybir.dt.int16
AF = mybir.ActivationFunctionType
OP = mybir.AluOpType
AX = mybir.AxisListType

B, N, L, K, CD, H, DH, D = 2, 2048, 1024, 16, 2, 4, 128, 128
HD = H * DH
FQ = 2.0
FV = 2.0
SCALE = 1.0 / float(np.sqrt(DH))
NCORES = 8
NQ = (B * N) // NCORES  # queries per core = 512
QT = NQ // 128  # query tiles per core = 4
NCH = 4  # chunks per query tile
CQ = 128 // NCH  # queries per chunk = 32
CR = CQ * K  # rows per chunk = 512
TR = 128 * K  # rows per tile = 2048
GELU = AF.Gelu_apprx_tanh
TWO_PI = 2.0 * np.pi

WSPECS = [
    ("rff", [CD, 128], F32),
    ("ws1", [128, 128], BF16),   # rows 0:64 eq_w1 sin-half, 64:128 ev_w1 sin-half
    ("wc1", [128, 128], BF16),   # rows 0:64 eq_w1 cos-half, 64:128 ev_w1 cos-half
    ("eq_b1", [128, 1], F32),
    ("Mq", [128, 512], BF16),
    ("w1vc", [128, 4], F32),
    ("w2v", [128, 4], BF16),
    ("attconst", [128, 1], F32),
    ("ev_b1", [128, 1], F32),
    ("ev_w2", [128, 128], BF16),
    ("ev_b2", [128, 1], F32),
    ("ivw1", [128, 128], BF16),
    ("ivb1", [128, 1], F32),
    ("ivw2g", [128, 512], BF16),
    ("wv", [128, 512], BF16),
    ("WA", [128, 512], BF16),
    ("WB", [128, 512], BF16),
    ("mw1", [128, 128], BF16),
    ("mb1p", [128, 4], F32),
    ("Wmo", [128, 512], BF16),
    ("wmo1m", [128, 512], BF16),  # row 32h, cols 128h:128h+128 = -(Wmo_h^T @ 1)
    ("bmo", [128, 1], F32),
    ("cw1", [128, 128], BF16),
    ("cb1", [128, 1], F32),
    ("cw2g", [128, 128], BF16),
    ("cw2b", [128, 128], BF16),
    ("cb2g1", [128, 1], F32),
    ("cb2b", [128, 1], F32),
]


def _bcast_inner(ap, n):
    """[.., Q] AP -> [.., Q, n] with a stride-0 inner dim (free broadcast)."""
    newap = [list(p) for p in ap.ap] + [[0, n]]
    return bass.AP(ap.tensor, ap.offset, newap)


def build_program():
    nc = bacc.Bacc()

    x_d = nc.declare_dram_parameter("x", [NQ, CD], F32, isOutput=False)
    xh_d = nc.declare_dram_parameter("xh", [NQ, DH], F32, isOutput=False)
    gtbl_d = nc.declare_dram_parameter("gtbl", [L, 3 * D], BF16, isOutput=False)
    p2t_d = nc.declare_dram_parameter("p2t", [CD, L], F32, isOutput=False)
    npsq_d = nc.declare_dram_parameter("npsq", [1, L], F32, isOutput=False)
    w_d = {}
    for name, shape, dt in WSPECS:
        w_d[name] = nc.declare_dram_parameter(name, shape, dt, isOutput=False)
    out_d = nc.declare_dram_parameter("out", [NQ, DH], F32, isOutput=True)

    with tile.TileContext(nc) as tc:
        _emit(nc, tc, x_d, xh_d, gtbl_d, p2t_d, npsq_d, w_d, out_d)
    nc.compile()
    return nc


def _emit(nc, tc, x_d, xh_d, gtbl_d, p2t_d, npsq_d, w_d, out_d):
    const = tc.alloc_tile_pool(name="const", bufs=1)
    wpool = tc.alloc_tile_pool(name="wpool", bufs=1)
    core = tc.alloc_tile_pool(name="core", bufs=1)
    hd = tc.alloc_tile_pool(name="hd", bufs=1)   # per-tile head tiles
    tl = tc.alloc_tile_pool(name="tl", bufs=1)   # per-tile body tiles
    ck = tc.alloc_tile_pool(name="ck", bufs=1)   # per-chunk transient tiles
    psp = tc.alloc_tile_pool(name="psp", bufs=1, space="PSUM")
    _pools = [const, wpool, core, hd, tl, ck, psp]

    _psn = [0]

    def PS(shape, tag, bufs, dtype=F32):
        _psn[0] += 1
        return psp.tile(
            shape, dtype, space="PSUM", tag=tag, bufs=bufs, name=f"ps_{tag}_{_psn[0]}"
        )

    # ---------- constants ----------
    ident = const.tile([128, 128], F32)
    make_identity(nc, ident[:])

    ones_col_bf = const.tile([128, 1], BF16)
    nc.vector.memset(ones_col_bf[:], 1.0)
    inv128_bf = const.tile([128, 1], BF16)
    nc.vector.memset(inv128_bf[:], 1.0 / 128.0)
    ones_row_bf = const.tile([1, 128], BF16)
    nc.vector.memset(ones_row_bf[:], 1.0)
    half_row128_bf = const.tile([1, 128], BF16)
    nc.vector.memset(half_row128_bf[:], 0.5)
    onesmat_bf = const.tile([128, 128], BF16)
    nc.vector.memset(onesmat_bf[:], 1.0)
    ones_row_f32 = const.tile([1, 128], F32)
    nc.vector.memset(ones_row_f32[:], 1.0)
    eps_col = const.tile([128, 1], F32)
    nc.vector.memset(eps_col[:], 1e-6)

    # ---------- weights ----------
    W = {}
    for name, shape, dt in WSPECS:
        wt = wpool.tile(shape, dt, name=f"w_{name}", tag=f"w_{name}")
        nc.sync.dma_start(out=wt[:], in_=w_d[name][:])
        W[name] = wt

    def Wh(name, h, w=128):
        return W[name][:, h * w : (h + 1) * w]

    # ---------- per-core precompute ----------
    x_sb = core.tile([128, QT, CD], F32)
    nc.sync.dma_start(out=x_sb[:], in_=x_d[:].rearrange("(t q) c -> q t c", q=128))
    xsq = core.tile([128, QT], F32)
    xs2 = core.tile([128, QT, CD], F32)
    nc.vector.tensor_tensor(out=xs2[:], in0=x_sb[:], in1=x_sb[:], op=OP.mult)
    nc.vector.tensor_reduce(out=xsq[:], in_=xs2[:], axis=AX.X, op=OP.add)

    x_fm = core.tile([CD, NQ], F32)
    for t in range(QT):
        tp = PS([CD, 128], "tr", 1)
        nc.tensor.transpose(out=tp[:], in_=x_sb[:, t, :], identity=ident[:])
        nc.vector.tensor_copy(out=x_fm[:, 128 * t : 128 * (t + 1)], in_=tp[:])

    p2_fm = core.tile([CD, L], F32)
    nc.sync.dma_start(out=p2_fm[:], in_=p2t_d[:])
    npsq = core.tile([1, L], F32)
    nc.sync.dma_start(out=npsq[:], in_=npsq_d[:])

    # ---------- cFFN on x_h (512 queries at once) ----------
    xh_fm = core.tile([128, NQ], BF16)
    xh_rm = core.tile([128, QT, DH], F32)
    nc.sync.dma_start(out=xh_rm[:], in_=xh_d[:].rearrange("(t q) c -> q t c", q=128))
    for t in range(QT):
        tp = PS([128, 128], "tr", 1)
        nc.tensor.transpose(out=tp[:], in_=xh_rm[:, t, :], identity=ident[:])
        nc.vector.tensor_copy(out=xh_fm[:, 128 * t : 128 * (t + 1)], in_=tp[:])

    c1ps = PS([128, NQ], "mm", 5)
    nc.tensor.matmul(out=c1ps[:], lhsT=W["cw1"][:], rhs=xh_fm[:], start=True, stop=True)
    c1 = ck.tile([128, NQ], BF16, tag="ev1", bufs=2)
    nc.scalar.activation(out=c1[:], in_=c1ps[:], func=GELU, bias=W["cb1"][:])

    cmean = PS([128, NQ], "stat", 2)
    nc.tensor.matmul(out=cmean[0:1, :], lhsT=inv128_bf[:], rhs=c1[:], start=True, stop=True)
    c1sq = ck.tile([128, NQ], BF16, tag="vg_sb", bufs=1)
    nc.vector.tensor_tensor(out=c1sq[:], in0=c1[:], in1=c1[:], op=OP.mult)
    cmsq = PS([128, NQ], "stat", 2)
    nc.tensor.matmul(out=cmsq[0:1, :], lhsT=inv128_bf[:], rhs=c1sq[:], start=True, stop=True)

    cm2 = core.tile([1, NQ], F32)
    nc.scalar.square(out=cm2[:], in_=cmean[0:1, :])
    cvar = core.tile([1, NQ], F32)
    nc.vector.tensor_tensor(out=cvar[:], in0=cmsq[0:1, :], in1=cm2[:], op=OP.subtract)
    crstd_bf = core.tile([1, NQ], BF16)
    nc.scalar.activation(
        out=crstd_bf[:], in_=cvar[:], func=AF.Abs_reciprocal_sqrt, bias=eps_col[0:1, :]
    )
    cmr_bf = core.tile([1, NQ], BF16)
    nc.vector.tensor_tensor(out=cmr_bf[:], in0=cmean[0:1, :], in1=crstd_bf[:], op=OP.mult)
    crbc = PS([128, NQ], "mm", 5)
    nc.tensor.matmul(out=crbc[:], lhsT=ones_row_bf[:], rhs=crstd_bf[:], start=True, stop=True)
    cmbc = PS([128, NQ], "mm", 5)
    nc.tensor.matmul(out=cmbc[:], lhsT=ones_row_bf[:], rhs=cmr_bf[:], start=True, stop=True)
    z1c = ck.tile([128, NQ], BF16, tag="z1", bufs=1)
    nc.vector.tensor_tensor(out=z1c[:], in0=c1[:], in1=crbc[:], op=OP.mult)
    zc = ck.tile([128, NQ], BF16, tag="p_sb", bufs=2)
    nc.vector.tensor_tensor(out=zc[:], in0=z1c[:], in1=cmbc[:], op=OP.subtract)

    gp1 = core.tile([128, NQ], BF16)
    modadd = core.tile([128, NQ], BF16)
    gps_ = PS([128, NQ], "mm", 5)
    nc.tensor.matmul(out=gps_[:], lhsT=W["cw2g"][:], rhs=zc[:], start=True, stop=True)
    nc.scalar.activation(out=gp1[:], in_=gps_[:], func=AF.Identity, bias=W["cb2g1"][:])
    btps = PS([128, NQ], "mm", 5)
    nc.tensor.matmul(out=btps[:], lhsT=W["cw2b"][:], rhs=zc[:], start=True, stop=True)
    bt = ck.tile([128, NQ], BF16, tag="ivsq", bufs=2)
    nc.scalar.activation(out=bt[:], in_=btps[:], func=AF.Identity, bias=W["cb2b"][:])
    ma1 = ck.tile([128, NQ], BF16, tag="zp", bufs=2)
    nc.vector.tensor_scalar(
        out=ma1[:], in0=gp1[:], scalar1=W["ev_b2"][:], scalar2=None, op0=OP.mult
    )
    nc.vector.tensor_tensor(out=modadd[:], in0=ma1[:], in1=bt[:], op=OP.add)

    # ================= per query tile: head phase =================
    # scores -> top-16 -> idx16 -> combined gather -> t_x frac
    def emit_head(t):
        qs = slice(128 * t, 128 * (t + 1))

        scores = hd.tile([128, L], F32, tag="scores", bufs=1, name=f"scores_{t}")
        for s in range(2):
            sl = slice(512 * s, 512 * (s + 1))
            scps = PS([128, 512], "mm", 5)
            nc.tensor.matmul(out=scps[:], lhsT=x_fm[:, qs], rhs=p2_fm[:, sl], start=True, stop=False)
            nc.tensor.matmul(out=scps[:], lhsT=ones_row_f32[:], rhs=npsq[:, sl], start=False, stop=True)
            nc.scalar.copy(out=scores[:, sl], in_=scps[:])
        vals = hd.tile([128, K], F32, tag="vals", bufs=2, name=f"vals_{t}")
        idxs = hd.tile([128, K], U32, tag="idxs", bufs=2, name=f"idxs_{t}")
        scr2 = hd.tile([128, L], F32, tag="scr2", bufs=1, name=f"scr2_{t}")
        nc.vector.max(out=vals[:, 0:8], in_=scores[:])
        nc.vector.max_index(out=idxs[:, 0:8], in_max=vals[:, 0:8], in_values=scores[:])
        nc.vector.match_replace(
            out=scr2[:], in_to_replace=vals[:, 0:8], in_values=scores[:], imm_value=-1e30
        )
        nc.vector.max(out=vals[:, 8:16], in_=scr2[:])
        nc.vector.max_index(out=idxs[:, 8:16], in_max=vals[:, 8:16], in_values=scr2[:])

        # index prep for dma_gather: idx16[k, q] (int16) replicated across
        # the 8 gpsimd cores' 16-partition blocks
        idxf = hd.tile([128, K], F32, tag="idxf", bufs=1, name=f"idxf_{t}")
        nc.vector.tensor_copy(out=idxf[:], in_=idxs[:])
        idxt_ps = PS([K, 128], "tr", 1)
        nc.tensor.transpose(out=idxt_ps[:], in_=idxf[:], identity=ident[:])
        idx16 = hd.tile([128, 128], I16, tag="idx16", bufs=2, name=f"idx16_{t}")
        nc.vector.tensor_copy(out=idx16[0:16, :], in_=idxt_ps[:])
        for b in range(1, 8):
            nc.sync.dma_start(out=idx16[16 * b : 16 * b + 16, :], in_=idx16[0:16, :])

        # ONE combined transposed gather for the whole tile (2048 rows):
        # G[128, 0, :] = c (bf16), G[128, 1, :] = frac(p@Bs) (f16 bits),
        # G[0, 2, :] = 1/sigma^2 (f16 bits)
        Gs = []
        for c in range(NCH):
            G = hd.tile([128, 3, CR], BF16, tag=f"G{c}", bufs=2, name=f"G{c}_{t}")
            nc.gpsimd.dma_gather(
                out_ap=G[:], in_ap=gtbl_d[:],
                idxs_ap=idx16[:, 32 * c : 32 * c + 32], num_idxs=CR,
                num_idxs_reg=CR, elem_size=3 * D, transpose=True,
            )
            Gs.append(G)

        # -d^2 row [1, 2048] (q-major) via sbuf->sbuf DMA flatten
        negd2 = hd.tile([128, K], BF16, tag="negd2", bufs=2, name=f"negd2_{t}")
        nc.vector.tensor_scalar(
            out=negd2[:], in0=vals[:], scalar1=xsq[:, t : t + 1], scalar2=None,
            op0=OP.subtract,
        )
        negd2_row = hd.tile([1, TR], BF16, tag="negd2_row", bufs=2, name=f"nd2r_{t}")
        nc.sync.dma_start(out=negd2_row[:], in_=negd2[:])

        # t_x = x @ Bs for this tile, pre-frac'd: txf = t_x - rint(t_x), f16
        txps = PS([128, 128], "tr", 1)
        nc.tensor.matmul(out=txps[:], lhsT=W["rff"][:], rhs=x_fm[:, qs], start=True, stop=True)
        t_x = hd.tile([128, 128], F32, tag="t_x", bufs=1, name=f"t_x_{t}")
        nc.vector.tensor_copy(out=t_x[:], in_=txps[:])
        txi = hd.tile([128, 128], I32, tag="txi", bufs=1, name=f"txi_{t}")
        nc.vector.tensor_copy(out=txi[:], in_=t_x[:])
        txf = hd.tile([128, 128], F32, tag="txf", bufs=2, name=f"txf_{t}")
        nc.vector.tensor_tensor(out=txf[:], in0=t_x[:], in1=txi[:], op=OP.subtract)
        return dict(Gs=Gs, negd2_row=negd2_row, txf=txf)

    # ================= per query tile: phased body =================
    # ACT table-set phases per steady-state iteration t:
    #   [silu]      sins(t) + mFFN gelus gms(t-1)     (Sin+Silu share a set)
    #   [gelu_tanh] g2/ev1/iv1 gelus (t) + att logits
    #   [absrsqrt]  iv-rstd(t) + m-rstd(t-1)
    #   [exp]       softmax att_e(t-1), then P10/P11(t-1) (no ACT sets)
    # Copy/Identity/Square/Abs are in every set (free).

    state = {}

    def emit_sins(t, ht):
        Gs = ht["Gs"]
        negd2_row = ht["negd2_row"]
        txf = ht["txf"]

        # ---- RFF frac chain (DVE) ----
        # tfull = txf - pbf in [-1, 1]; range-wrap replaces the int-cast
        # round: fs = wrap(tfull) in [-.5, .5]; fc = wrap(tfull + .25) so
        # sin(2*pi*fc) = cos(2*pi*tfull) with no abs / bias needed.
        fs = tl.tile([128, TR], F16, tag="fs", bufs=1, name=f"fs_{t}")
        fc = tl.tile([128, TR], F16, tag="fc", bufs=1, name=f"fc_{t}")
        for c in range(NCH):
            cs = slice(CR * c, CR * (c + 1))
            tfc = ck.tile([128, CR], F16, tag="tfc", bufs=2)
            nc.vector.tensor_tensor(
                out=tfc[:].rearrange("p (a b) -> p a b", a=CQ),
                in0=_bcast_inner(txf[:, CQ * c : CQ * (c + 1)], K),
                in1=Gs[c][:, 1, :].bitcast(F16).rearrange("p (a b) -> p a b", a=CQ),
                op=OP.subtract,
            )
            nc.vector.add_range_wrap(out=fs[:, cs], in_=tfc[:], shift=0.0, bound=0.5, period=1.0)
            nc.vector.add_range_wrap(out=fc[:, cs], in_=tfc[:], shift=0.25, bound=0.5, period=1.0)
        # gw row [1, 2048] = -d^2 * (1/sigma^2)
        gw_row = tl.tile([1, TR], BF16, tag="gw_row", bufs=1, name=f"gw_{t}")
        for c in range(NCH):
            cs = slice(CR * c, CR * (c + 1))
            nc.vector.tensor_tensor(
                out=gw_row[:, cs], in0=negd2_row[:, cs],
                in1=Gs[c][0:1, 2, :].bitcast(F16), op=OP.mult,
            )

        # ---- sins (silu set) ----
        sfull = tl.tile([128, TR], BF16, tag="sfull", bufs=1, name=f"sfull_{t}")
        cfull = tl.tile([128, TR], BF16, tag="cfull", bufs=1, name=f"cfull_{t}")
        nc.scalar.activation(out=sfull[:], in_=fs[:], func=AF.Sin, scale=TWO_PI)
        nc.scalar.activation(out=cfull[:], in_=fc[:], func=AF.Sin, scale=TWO_PI)
        st = state.setdefault(t, {})
        st.update(sfull=sfull, cfull=cfull, gw_row=gw_row, Gs=Gs)

    def emit_geluA(t):
        st = state[t]
        sfull, cfull, gw_row, Gs = st["sfull"], st["cfull"], st["gw_row"], st["Gs"]

        def cg(c):
            return Gs[c][:, 0, :]

        # ---- q path + attention logits ----
        # all in-loop gelus use the sigmoid approx via Silu so they share
        # one ACT table set with Sin; consumer weights absorb 1/1.702
        g2 = tl.tile([128, TR], BF16, tag="g2", bufs=1, name=f"g2_{t}")
        att_l = tl.tile([128, TR], BF16, tag="att_l", bufs=1, name=f"attl_{t}")
        iv1 = tl.tile([128, TR], BF16, tag="iv1", bufs=1, name=f"iv1_{t}")
        # ivs33 rows: 0 = irstd (later), 32 = imr (later)
        ivs33 = tl.tile([33, TR], BF16, tag="ivs33", bufs=1, name=f"ivs33_{t}")
        ivmean_row = tl.tile([1, TR], BF16, tag="ivmean_row", bufs=1, name=f"ivmr_{t}")
        ivar_all = tl.tile([1, TR], BF16, tag="ivar_all", bufs=1, name=f"ivva_{t}")
        for c in range(NCH):
            cs = slice(CR * c, CR * (c + 1))
            gq = slice(128 * t + CQ * c, 128 * t + CQ * (c + 1))
            # g2 (rows 0:64) and ev1 (rows 64:128) interleaved: disjoint
            # row-groups let the PE overlap the pairs
            g2ps = PS([128, CR], "mm", 5)
            ev1ps = PS([128, CR], "mm", 5)
            nc.tensor.matmul(out=g2ps[:], lhsT=W["ws1"][0:64, :], rhs=sfull[0:64, cs], start=True, stop=False)
            nc.tensor.matmul(out=ev1ps[:], lhsT=W["ws1"][64:128, :], rhs=sfull[64:128, cs], start=True, stop=False)
            nc.tensor.matmul(out=g2ps[:], lhsT=W["wc1"][0:64, :], rhs=cfull[0:64, cs], start=False, stop=True)
            nc.tensor.matmul(out=ev1ps[:], lhsT=W["wc1"][64:128, :], rhs=cfull[64:128, cs], start=False, stop=True)
            nc.scalar.activation(out=g2[:, cs], in_=g2ps[:], func=AF.Silu, scale=1.702, bias=W["eq_b1"][:])
            ev1 = ck.tile([128, CR], BF16, tag="ev1", bufs=2)
            nc.scalar.activation(out=ev1[:], in_=ev1ps[:], func=AF.Silu, scale=1.702, bias=W["ev_b1"][:])

            # attention: 4 full MMs evacuated by ScalarE, qkp in-place on
            # the evac tile (both-SBUF bf16 -> 2x DVE mode), then the 8
            # small reduce MMs back-to-back on distinct col-groups
            attps = PS([128, CR], "stat", 2)
            nc.tensor.matmul(
                out=attps[:], lhsT=half_row128_bf[:],
                rhs=gw_row[:, cs], start=True, stop=False,
                skip_group_check=True,
            )
            qkp = ck.tile([128, H, CR], BF16, tag="qkp", bufs=1)
            for h in range(H):
                ups = PS([128, CR], "mm", 5)
                nc.tensor.matmul(out=ups[:], lhsT=Wh("Mq", h), rhs=g2[:, cs], start=True, stop=True)
                # w1v (= wk_h @ bq_h) is a per-partition constant: apply it
                # as the evacuation bias instead of a broadcast matmul
                nc.scalar.activation(
                    out=qkp[:, h, :], in_=ups[:], func=AF.Identity,
                    bias=W["w1vc"][:, h : h + 1],
                )
            for h in range(H):
                nc.vector.tensor_tensor(out=qkp[:, h, :], in0=qkp[:, h, :], in1=cg(c), op=OP.mult)
            for h in range(H):
                nc.tensor.matmul(
                    out=attps[32 * h : 32 * h + 1, :], lhsT=ones_col_bf[:], rhs=qkp[:, h, :],
                    start=False, stop=False, tile_position=(0, 32 * h),
                    skip_group_check=True,
                )
            for h in range(H):
                nc.tensor.matmul(
                    out=attps[32 * h : 32 * h + 1, :], lhsT=W["w2v"][:, h : h + 1],
                    rhs=g2[:, cs], start=False, stop=(h == H - 1), tile_position=(0, 32 * h),
                    skip_group_check=True,
                )
            nc.scalar.copy(out=att_l[:, cs], in_=attps[:])

            # ---- ev modulation -> iv1 ----
            ev2ps = PS([128, CR], "mm", 5)
            nc.tensor.matmul(out=ev2ps[:], lhsT=W["ev_w2"][:], rhs=ev1[:], start=True, stop=True)
            mv = ck.tile([128, CQ, K], BF16, tag="mv", bufs=1)
            nc.vector.tensor_tensor(
                out=mv[:], in0=ev2ps[:].rearrange("p (a b) -> p a b", a=CQ),
                in1=_bcast_inner(gp1[:, gq], K), op=OP.mult,
            )
            ivin = ck.tile([128, CQ, K], BF16, tag="ivin", bufs=2)
            nc.vector.tensor_tensor(
                out=ivin[:], in0=mv[:], in1=_bcast_inner(modadd[:, gq], K), op=OP.add
            )
            iv1ps = PS([128, CR], "mm", 5)
            nc.tensor.matmul(
                out=iv1ps[:], lhsT=W["ivw1"][:],
                rhs=ivin[:].rearrange("p a b -> p (a b)"), start=True, stop=True,
            )
            nc.scalar.activation(out=iv1[:, cs], in_=iv1ps[:], func=AF.Silu, scale=1.702, bias=W["ivb1"][:])

            # ---- iv LN stats (PE + in-set Square/Copy); rstd deferred ----
            ivst = PS([128, CR], "stat", 2)
            ivsq = ck.tile([128, CR], BF16, tag="ivsq", bufs=2)
            nc.vector.tensor_tensor(out=ivsq[:], in0=iv1[:, cs], in1=iv1[:, cs], op=OP.mult)
            nc.tensor.matmul(
                out=ivst[0:1, :], lhsT=inv128_bf[:], rhs=iv1[:, cs],
                start=True, stop=True, skip_group_check=True,
            )
            nc.tensor.matmul(
                out=ivst[32:33, :], lhsT=inv128_bf[:], rhs=ivsq[:],
                start=True, stop=True, tile_position=(0, 32),
                skip_group_check=True,
            )
            # mean parked at ivs33 row 16 (DVE handles the partition shift)
            nc.vector.tensor_copy(out=ivmean_row[:, cs], in_=ivst[0:1, :])
            im2 = ck.tile([1, CR], BF16, tag="im2")
            nc.scalar.square(out=im2[:], in_=ivst[0:1, :])
            nc.vector.tensor_tensor(
                out=ivar_all[:, cs], in0=ivst[32:33, :], in1=im2[:], op=OP.subtract
            )
        st.update(g2=g2, att_l=att_l, iv1=iv1, ivs33=ivs33, ivmean_row=ivmean_row, ivar_all=ivar_all)

    def emit_ivrstd(t):
        # [absrsqrt set] one rstd for the whole tile, then LN apply (DVE/PE)
        st = state[t]
        iv1, ivs33, ivar_all = st["iv1"], st["ivs33"], st["ivar_all"]
        ivmean_row = st["ivmean_row"]
        nc.scalar.activation(
            out=ivs33[0:1, :], in_=ivar_all[:], func=AF.Abs_reciprocal_sqrt,
            bias=eps_col[0:1, :],
        )
        # imr at partition 32 so its broadcast MM uses row-group 1 (overlaps
        # with irbc's row-group 0)
        nc.vector.tensor_tensor(
            out=ivs33[32:33, :], in0=ivmean_row[:], in1=ivs33[0:1, :], op=OP.mult
        )
        ziv = tl.tile([128, TR], BF16, tag="ziv", bufs=1, name=f"ziv_{t}")
        for c in range(NCH):
            cs = slice(CR * c, CR * (c + 1))
            irbc = PS([128, CR], "mm", 5)
            imbc = PS([128, CR], "mm", 5)
            nc.tensor.matmul(out=irbc[:], lhsT=ones_row_bf[:], rhs=ivs33[0:1, cs], start=True, stop=True)
            nc.tensor.matmul(
                out=imbc[:], lhsT=onesmat_bf[32:33, :], rhs=ivs33[32:33, cs],
                start=True, stop=True, tile_position=(32, 0),
            )
            z1 = ck.tile([128, CR], BF16, tag="z1", bufs=1)
            nc.vector.tensor_tensor(out=z1[:], in0=iv1[:, cs], in1=irbc[:], op=OP.mult)
            nc.vector.tensor_tensor(out=ziv[:, cs], in0=z1[:], in1=imbc[:], op=OP.subtract)
        st["ziv"] = ziv

    def emit_gms(t):
        # [silu set] mFFN: gelu via silu(1.702x) (=1.702*gelu_sig(x); the
        # 1.702 factor cancels exactly through the LN rstd)
        st = state[t]
        ziv, Gs = st["ziv"], st["Gs"]

        def cg(c):
            return Gs[c][:, 0, :]

        gms = [
            tl.tile([128, TR], BF16, tag=f"gm{h}", bufs=1, name=f"gm{h}_{t}")
            for h in range(H)
        ]
        mvar_all = tl.tile([128, TR], BF16, tag="mvar", bufs=1, name=f"mvar_{t}")
        mmean_sb = tl.tile([128, TR], BF16, tag="mmean_sb", bufs=1, name=f"mmean_{t}")
        for c in range(NCH):
            cs = slice(CR * c, CR * (c + 1))
            mmean = PS([128, CR], "stat", 2)
            msqp = PS([128, CR], "stat", 2)
            gsq = ck.tile([128, H, CR], BF16, tag="gsq", bufs=1)
            for h in range(H):
                vgps = PS([128, CR], "mm", 5)
                nc.tensor.matmul(out=vgps[:], lhsT=Wh("ivw2g", h), rhs=ziv[:, cs], start=True, stop=True)
                vg_sb = ck.tile([128, CR], BF16, tag="vg_sb", bufs=1)
                nc.scalar.copy(out=vg_sb[:], in_=vgps[:])
                v0ps = PS([128, CR], "mm", 5)
                nc.tensor.matmul(out=v0ps[:], lhsT=Wh("wv", h), rhs=cg(c), start=True, stop=True)
                p_sb = ck.tile([128, CR], BF16, tag="p_sb", bufs=2)
                nc.scalar.copy(out=p_sb[:], in_=v0ps[:])
                nc.vector.tensor_tensor(out=p_sb[:], in0=p_sb[:], in1=vg_sb[:], op=OP.mult)
                m1ps = PS([128, CR], "mm", 5)
                nc.tensor.matmul(out=m1ps[:], lhsT=W["mw1"][:], rhs=p_sb[:], start=True, stop=False)
                nc.tensor.matmul(out=m1ps[:], lhsT=Wh("WA", h), rhs=cg(c), start=False, stop=False)
                nc.tensor.matmul(out=m1ps[:], lhsT=Wh("WB", h), rhs=ziv[:, cs], start=False, stop=True)
                nc.scalar.activation(
                    out=gms[h][:, cs], in_=m1ps[:], func=AF.Silu, scale=1.702,
                    bias=W["mb1p"][:, h : h + 1],
                )
                nc.vector.tensor_tensor(out=gsq[:, h, :], in0=gms[h][:, cs], in1=gms[h][:, cs], op=OP.mult)
            # stat MMs in two 4-bursts (col-groups 0..3 back-to-back)
            for h in range(H):
                nc.tensor.matmul(
                    out=mmean[32 * h : 32 * h + 1, :], lhsT=inv128_bf[:], rhs=gms[h][:, cs],
                    start=True, stop=True, tile_position=(0, 32 * h),
                    skip_group_check=True,
                )
            for h in range(H):
                nc.tensor.matmul(
                    out=msqp[32 * h : 32 * h + 1, :], lhsT=inv128_bf[:], rhs=gsq[:, h, :],
                    start=True, stop=True, tile_position=(0, 32 * h),
                    skip_group_check=True,
                )
            mm2 = ck.tile([128, CR], BF16, tag="mm2")
            nc.scalar.square(out=mm2[:], in_=mmean[:])
            nc.vector.tensor_tensor(out=mvar_all[:, cs], in0=msqp[:], in1=mm2[:], op=OP.subtract)
            nc.scalar.copy(out=mmean_sb[:, cs], in_=mmean[:])
        st.update(gms=gms, mvar_all=mvar_all, mmean_sb=mmean_sb)

    def emit_mrstd(t):
        # [absrsqrt set] — in place: mvar_all becomes mrstd, mmean_sb becomes mmr
        st = state[t]
        mrstd = st["mvar_all"]
        mmr = st["mmean_sb"]
        nc.scalar.activation(
            out=mrstd[:], in_=mrstd[:], func=AF.Abs_reciprocal_sqrt,
            bias=eps_col[:],
        )
        nc.vector.tensor_tensor(out=mmr[:], in0=mmr[:], in1=mrstd[:], op=OP.mult)
        st.update(mrstd=mrstd, mmr=mmr)

    def emit_exp(t):
        # [exp set] softmax numerator
        st = state[t]
        att_e = tl.tile([128, TR], BF16, tag="att_e", bufs=1, name=f"atte_{t}")
        nc.scalar.activation(out=att_e[:], in_=st["att_l"][:], func=AF.Exp, bias=W["attconst"][:])
        st["att_e"] = att_e

    def emit_out(t):
        # P10/P11 (no new ACT sets: Identity only)
        st = state[t]
        att_e, mrstd, mmr, gms = st["att_e"], st["mrstd"], st["mmr"], st["gms"]
        den_t = tl.tile([128, 128], F32, tag="den_t", name=f"den_{t}")
        nc.vector.tensor_reduce(
            out=den_t[:], in_=att_e[:].rearrange("p (a b) -> p a b", a=128),
            axis=AX.X, op=OP.add,
        )
        rden_t = tl.tile([128, 128], F32, tag="rden_t", name=f"rden_{t}")
        nc.vector.reciprocal(out=rden_t[:], in_=den_t[:])
        rdbf = tl.tile([128, 128], BF16, tag="rdbf", name=f"rdbf_{t}")
        nc.vector.tensor_copy(out=rdbf[:], in_=rden_t[:])

        a2 = tl.tile([128, TR], BF16, tag="a2", name=f"a2_{t}")
        nc.vector.tensor_tensor(out=a2[:], in0=att_e[:], in1=mrstd[:], op=OP.mult)
        # a3 in place over att_e (att_e's last readers are den and a2)
        a3 = att_e
        nc.vector.tensor_tensor(out=a3[:], in0=att_e[:], in1=mmr[:], op=OP.mult)
        s3 = tl.tile([128, 128], F32, tag="s3", name=f"s3_{t}")
        nc.vector.tensor_reduce(
            out=s3[:], in_=a3[:].rearrange("p (a b) -> p a b", a=128), axis=AX.X, op=OP.add
        )
        s3rb = tl.tile([128, 128], BF16, tag="s3rb", name=f"s3rb_{t}")
        nc.vector.tensor_tensor(out=s3rb[:], in0=s3[:], in1=rden_t[:], op=OP.mult)

        # fold 1/den into a2 (in place, k-broadcast) so zacc needs no rescale
        nc.vector.tensor_tensor(
            out=a2[:].rearrange("p (a b) -> p a b", a=128),
            in0=a2[:].rearrange("p (a b) -> p a b", a=128),
            in1=_bcast_inner(rdbf[:, :], K), op=OP.mult,
        )

        zaccb = [
            tl.tile([128, 128], BF16, tag=f"zaccb{h}", name=f"zaccb{h}_{t}", bufs=2)
            for h in range(H)
        ]
        zaccs = [
            tl.tile([128, 128], F32, tag=f"zacc{h}", name=f"zacc{h}_{t}")
            for h in range(H)
        ]
        for c in range(NCH):
            cs = slice(CR * c, CR * (c + 1))
            qsl = slice(CQ * c, CQ * (c + 1))
            # 4 broadcast MMs back-to-back (row-groups 0..3: concurrent)
            a2bcs = []
            for h in range(H):
                a2bc = PS([128, CR], "mm", 5)
                nc.tensor.matmul(
                    out=a2bc[:], lhsT=onesmat_bf[32 * h : 32 * h + 1, :],
                    rhs=a2[32 * h : 32 * h + 1, cs], start=True, stop=True,
                    tile_position=(32 * h, 0),
                )
                a2bcs.append(a2bc)
            for h in range(H):
                zp = ck.tile([128, CR], BF16, tag="zp", bufs=2)
                nc.scalar.copy(out=zp[:], in_=a2bcs[h][:])
                nc.vector.tensor_tensor(out=zp[:], in0=gms[h][:, cs], in1=zp[:], op=OP.mult)
                nc.vector.tensor_reduce(
                    out=zaccs[h][:, qsl], in_=zp[:].rearrange("p (a b) -> p a b", a=CQ),
                    axis=AX.X, op=OP.add,
                )
        for h in range(H):
            nc.vector.tensor_copy(out=zaccb[h][:], in_=zaccs[h][:])

        outps = PS([128, 128], "tr", 1)
        for h in range(H):
            nc.tensor.matmul(
                out=outps[:], lhsT=Wh("Wmo", h), rhs=zaccb[h][:], start=(h == 0), stop=False,
                skip_group_check=True,
            )
            nc.tensor.matmul(
                out=outps[:], lhsT=W["wmo1m"][32 * h : 32 * h + 1, 128 * h : 128 * (h + 1)],
                rhs=s3rb[32 * h : 32 * h + 1, :], start=False, stop=(h == H - 1),
                tile_position=(32 * h, 0), skip_group_check=True,
            )
        outsb = tl.tile([128, 128], F32, tag="outsb")
        nc.scalar.activation(out=outsb[:], in_=outps[:], func=AF.Identity, bias=W["bmo"][:])
        trp = PS([128, 128], "tr", 1)
        nc.tensor.transpose(out=trp[:], in_=outsb[:], identity=ident[:])
        outrm = tl.tile([128, 128], F32, tag="outrm")
        nc.vector.tensor_copy(out=outrm[:], in_=trp[:])
        nc.sync.dma_start(out=out_d[slice(128 * t, 128 * (t + 1)), :], in_=outrm[:])
        del state[t]

    # ---- pipelined emission ----
    # iteration t: [silu: sins(t), gms(t-1)] [exp: att_e(t-1)]
    #             [gelu_tanh: geluA(t)] [absrsqrt: ivr(t), mr(t-1)]
    #             then out(t-1) (no ACT sets) + head(t+2)
    heads = {}
    heads[0] = emit_head(0)
    heads[1] = emit_head(1)
    for t in range(QT):
        emit_sins(t, heads[t])
        del heads[t]
        if t + 2 < QT:
            heads[t + 2] = emit_head(t + 2)
        if t - 1 >= 0:
            emit_gms(t - 1)
            emit_exp(t - 1)
        emit_geluA(t)
        emit_ivrstd(t)
        if t - 1 >= 0:
            emit_mrstd(t - 1)
            emit_out(t - 1)
    # tail: tile QT-1
    emit_gms(QT - 1)
    emit_exp(QT - 1)
    emit_mrstd(QT - 1)
    emit_out(QT - 1)

    for p in reversed(_pools):
        p.release()


# ======================= host side =======================


def _host_prep(inputs):
    f = {k: np.asarray(v, np.float32) for k, v in inputs.items()}

    def bf(x):
        return np.ascontiguousarray(np.asarray(x, np.float32)).astype(ml_dtypes.bfloat16)

    def col(x):
        return np.ascontiguousarray(np.asarray(x, np.float32).reshape(-1, 1))

    rff = np.concatenate([FQ * f["rffq"], FV * f["rffv"]], axis=1)  # [2,128]

    wq_s = f["wq"] * SCALE
    bq_s = f["bq"] * SCALE
    W_qm = f["eq_w2"] @ wq_s
    b_qm = f["eq_b2"] @ wq_s + bq_s
    Mq = np.zeros((128, 512), np.float32)
    w1vc = np.zeros((128, 4), np.float32)
    w2v = np.zeros((128, 4), np.float32)
    attconst = np.zeros((128, 1), np.float32)
    for h in range(H):
        sl = slice(128 * h, 128 * (h + 1))
        Wq_h = W_qm[:, sl]
        wk_h = f["wk"][:, sl]
        bk_h = f["bk"][sl]
        bq_h = b_qm[sl]
        Mq[:, sl] = Wq_h @ wk_h.T
        w1vc[:, h] = wk_h @ bq_h
        w2v[:, h] = Wq_h @ bk_h
        attconst[32 * h, 0] = float(bq_h @ bk_h)

    # split eq_w1 / ev_w1 into sin/cos input halves
    ws1 = np.zeros((128, 128), np.float32)
    wc1 = np.zeros((128, 128), np.float32)
    ws1[0:64, :] = f["eq_w1"][0:64, :]
    wc1[0:64, :] = f["eq_w1"][64:128, :]
    ws1[64:128, :] = f["ev_w1"][0:64, :]
    wc1[64:128, :] = f["ev_w1"][64:128, :]

    ivw2f = f["ivls"][:, None] * f["ivw2"]
    ivb2f = f["ivb2"] + f["ivlb"] @ f["ivw2"]
    ivw2g = ivw2f[:, :HD]
    ivw2b = ivw2f[:, HD:]
    # bilinear expansion: m1 = mw1.T (v0*vg) + WA.T cg + WB.T ziv + mb1p
    WA = np.zeros((128, 512), np.float32)
    WB = np.zeros((128, 512), np.float32)
    mb1p = np.zeros((128, H), np.float32)
    for h in range(H):
        sl = slice(128 * h, 128 * (h + 1))
        c1_h = 1.0 + ivb2f[:HD][sl]
        bv_h = f["bv"][sl]
        b2_h = ivb2f[HD:][sl]
        WA[:, sl] = f["wv"][:, sl] @ np.diag(c1_h) @ f["mw1"]
        WB[:, sl] = (ivw2g[:, sl] @ np.diag(bv_h) + ivw2b[:, sl]) @ f["mw1"]
        mb1p[:, h] = f["mb1"] + (bv_h * c1_h + b2_h) @ f["mw1"]

    mw2f = f["mls"][:, None] * f["mw2"]
    mb2f = f["mb2"] + f["mlb"] @ f["mw2"]
    Wmo = np.zeros((128, 512), np.float32)
    wmo1m = np.zeros((128, 512), np.float32)
    for h in range(H):
        wo_h = f["wo"][128 * h : 128 * (h + 1), :]
        Wmo_h = mw2f @ wo_h
        Wmo[:, 128 * h : 128 * (h + 1)] = Wmo_h
        wmo1m[32 * h, 128 * h : 128 * (h + 1)] = -Wmo_h.sum(axis=0)
    bmo = f["bo"] + sum(mb2f @ f["wo"][128 * h : 128 * (h + 1), :] for h in range(H))

    cw2f = f["cls"][:, None] * f["cw2"]
    cb2f = f["cb2"] + f["clb"] @ f["cw2"]

    weights = {
        "rff": np.ascontiguousarray(rff),
        "ws1": bf(ws1),
        "wc1": bf(wc1),
        "eq_b1": col(1.702 * f["eq_b1"]),
        "Mq": bf(Mq / 1.702),
        "w1vc": np.ascontiguousarray(w1vc),
        "w2v": bf(w2v / 1.702),
        "attconst": attconst.astype(np.float32),
        "ev_b1": col(1.702 * f["ev_b1"]),
        "ev_w2": bf(f["ev_w2"] / 1.702),
        "ev_b2": col(f["ev_b2"]),
        "ivw1": bf(f["ivw1"]),
        "ivb1": col(1.702 * f["ivb1"]),
        "ivw2g": bf(ivw2g),
        "wv": bf(f["wv"]),
        "WA": bf(WA),
        "WB": bf(WB),
        "mw1": bf(f["mw1"]),
        # mFFN gelu is computed as silu(1.702*(m1+mb1p)) = 1.702*gelu_sig(m1+mb1p)
        # (the 1.702 factor cancels through the LN rstd), so pre-scale the bias
        "mb1p": np.ascontiguousarray(1.702 * mb1p),
        "Wmo": bf(Wmo),
        "wmo1m": bf(wmo1m),
        "bmo": col(bmo),
        "cw1": bf(f["cw1"]),
        "cb1": col(f["cb1"]),
        "cw2g": bf(cw2f[:, :DH]),
        "cw2b": bf(cw2f[:, DH:]),
        "cb2g1": col(cb2f[:DH] + 1.0),
        "cb2b": col(cb2f[DH:]),
    }

    x_flat = f["x"].reshape(B * N, CD)
    xh_flat = f["x_h"].reshape(B * N, DH)

    in_maps = []
    for i in range(NCORES):
        b = (i * NQ) // N
        rs = slice(i * NQ, (i + 1) * NQ)
        p_b = f["p"][b]
        c_b = f["c"][b]
        sig_b = f["window_sigma"][b]
        inv2 = (1.0 / (sig_b[:, 0] ** 2)).astype(np.float16)
        pb = (p_b @ rff).astype(np.float32)
        pbf = (pb - np.rint(pb)).astype(np.float16)
        gtbl = np.zeros((L, 3 * D), ml_dtypes.bfloat16)
        gtbl[:, :D] = bf(c_b)
        gtbl[:, D : 2 * D] = pbf.view(ml_dtypes.bfloat16)
        gtbl[:, 2 * D] = inv2.view(ml_dtypes.bfloat16)
        m = {
            "x": np.ascontiguousarray(x_flat[rs]),
            "xh": np.ascontiguousarray(xh_flat[rs]),
            "gtbl": gtbl,
            "p2t": np.ascontiguousarray((2.0 * p_b).T),
            "npsq": np.ascontiguousarray(-(p_b**2).sum(1)[None, :]),
        }
        m.update(weights)
        in_maps.append(m)
    return in_maps


_PROGRAM_CACHE = {}


def kernel(**inputs):
    in_maps = _host_prep(inputs)
    if "nc" not in _PROGRAM_CACHE:
        _PROGRAM_CACHE["nc"] = build_program()
    nc = _PROGRAM_CACHE["nc"]

    from concourse.bass_utils import run_bass_kernel_spmd

    res = run_bass_kernel_spmd(nc, in_maps, core_ids=list(range(NCORES)))
    outs = [np.asarray(res.results[i]["out"], np.float32) for i in range(NCORES)]
    return np.concatenate(outs, axis=0).reshape(B, N, DH)

